# revision 1
# baseline (speedup 1.0000x reference)
"""Trainium2 Bass kernel for nn_LongTermEncoder (gnn_message_passing).

Sharding: data-parallel over batch B=8 across 8 NeuronCores (adjacency +
params replicated). The device kernel computes, per layer, the node-dim
message passing (the 4 dense [1000x1000] adjacency applies) that dominates
the FLOPs/memory; host (numpy, exactly validated vs the jax reference)
computes the graph constructor, inception convs, channel projections,
layernorm and pooling. mixprop is refactored exactly:

  out = Q0 x + A(Q1 x + A(Q2 x)) + B(R1 x + B(R2 x)),  A=(adp+I)/2,
  B = D^-1(adp^T+I);  channel mixing (Qk) commutes with node mixing (A).
"""
import math
import numpy as np

L, GDEP, PA, ALPHA, KTOP, TSHORT, EPS = 3, 2, 0.05, 3.0, 20, 12, 1e-5
KSET = (2, 4, 6, 8)
N, B, RC, CC = 1000, 8, 8, 32
TP = 161          # padded per-layer output T (layer T': 161/154/147)
F = RC * TP       # 1288 free elems per 8-channel block
f32 = np.float32


# ---------------- host math (validated vs reference) ----------------
def _graph_prep(d):
    emb1, emb2 = d["emb1"], d["emb2"]
    v1 = np.tanh(ALPHA * (emb1 @ d["lin1_w"].T + d["lin1_b"])).astype(f32)
    v2 = np.tanh(ALPHA * (emb2 @ d["lin2_w"].T + d["lin2_b"])).astype(f32)
    a = v1 @ v2.T - v2 @ v1.T
    adj = np.maximum(np.tanh(ALPHA * a), 0.0).astype(f32)
    score = adj + f32(0.01) * d["topk_noise"]
    t1 = np.argsort(-score, axis=1, kind="stable")[:, :KTOP]
    mask = np.zeros((N, N), f32)
    np.put_along_axis(mask, t1, 1.0, axis=1)
    adp = adj * mask
    mv = (1.0 - d["cooldowns"]).astype(f32)
    z = adp * (mv[:, None] * mv[None, :])
    z = z - z.max(axis=1, keepdims=True)
    e = np.exp(z)
    return (e / e.sum(axis=1, keepdims=True)).astype(f32)


def _fold(d, l):
    W = d["g1_w"][l]
    W0, W1, W2 = W[:, :32], W[:, 32:64], W[:, 64:]
    V = d["g2_w"][l]
    V0, V1, V2 = V[:, :32], V[:, 32:64], V[:, 64:]
    al, g = PA, 1.0 - PA
    Q0 = W0 + al * W1 + al * W2
    Q1 = g * W1 + g * al * W2
    Q2 = g * g * W2
    R0 = V0 + al * V1 + al * V2
    R1 = g * V1 + g * al * V2
    R2 = g * g * V2
    ub = d["g1_b"][l] + d["g2_b"][l]
    return Q0, Q1, Q2, R0, R1, R2, ub


def _conv_branch(x, w, b, Tp):
    k = w.shape[-1]
    T = x.shape[-1]
    out = np.zeros((w.shape[0], x.shape[1], T - k + 1), f32)
    for j in range(k):
        out += np.einsum("oi,int->ont", w[:, :, 0, j], x[:, :, j:T - k + 1 + j])
    return (out + b[:, None, None])[..., -Tp:]


def _host_apply(adp, dinv, p0, p1, p2, q1, q2):
    # exact host fallback of the device computation
    Ahalf = adp
    z = np.einsum("vw,bowt->bovt", Ahalf, p2)
    s1 = 0.5 * (z + p2) + p1
    z1 = np.einsum("vw,bowt->bovt", Ahalf, s1)
    u = p0 + 0.5 * (z1 + s1)
    zz = np.einsum("wv,bowt->bovt", adp, q2)
    s1b = dinv[None, None, :, None] * (zz + q2) + q1
    zz1 = np.einsum("wv,bowt->bovt", adp, s1b)
    u = u + dinv[None, None, :, None] * (zz1 + s1b)
    return u.astype(f32)


# ---------------- device kernel ----------------
_DEV = {"nc": None, "fail": False}


def _build_nc():
    import concourse.bass as bass
    import concourse.mybir as mybir
    from concourse.tile import TileContext

    bf = mybir.dt.bfloat16
    fp = mybir.dt.float32
    nc = bass.Bass()
    pn_d = nc.declare_dram_parameter("pn", (N, 4 * F), bf, isOutput=False)
    p0_d = nc.declare_dram_parameter("p0", (N, F), fp, isOutput=False)
    adpT_d = nc.declare_dram_parameter("adpT", (N, N), bf, isOutput=False)
    adp_d = nc.declare_dram_parameter("adp", (N, N), bf, isOutput=False)
    dinv_d = nc.declare_dram_parameter("dinv", (128, 8), fp, isOutput=False)
    u_d = nc.declare_dram_parameter("u", (N, F), fp, isOutput=True)

    NT = 8                       # node tiles
    rows = [128] * 7 + [104]
    off = [128 * i for i in range(NT)]
    CH = (512, 512, 264)         # free chunks of F=1288
    coff = (0, 512, 1024)
    MUL = mybir.AluOpType.mult
    ADD = mybir.AluOpType.add

    with TileContext(nc) as tc:
        with tc.tile_pool(name="res", bufs=1) as res, \
             tc.tile_pool(name="wk", bufs=3) as wk, \
             tc.tile_pool(name="ps", bufs=4, space="PSUM") as psp:
            aT, aD, pn, s1, s2 = [], [], [], [], []
            for k in range(NT):
                t = res.tile([128, N], bf, tag=f"aT{k}", name=f"aT{k}")
                nc.sync.dma_start(out=t[:rows[k], :], in_=adpT_d[off[k]:off[k] + rows[k], :])
                aT.append(t)
                t = res.tile([128, N], bf, tag=f"aD{k}", name=f"aD{k}")
                nc.sync.dma_start(out=t[:rows[k], :], in_=adp_d[off[k]:off[k] + rows[k], :])
                aD.append(t)
                t = res.tile([128, 4 * F], bf, tag=f"pn{k}", name=f"pn{k}")
                nc.sync.dma_start(out=t[:rows[k], :], in_=pn_d[off[k]:off[k] + rows[k], :])
                pn.append(t)
                s1.append(res.tile([128, F], bf, tag=f"s1{k}", name=f"s1{k}"))
                s2.append(res.tile([128, F], bf, tag=f"s2{k}", name=f"s2{k}"))
            dv = res.tile([128, 8], fp, tag="dinv")
            nc.sync.dma_start(out=dv[:, :], in_=dinv_d[:, :])

            # block column offsets in pn: [p2 | m1 | q2 | q1]
            P2, M1, Q2, Q1 = 0, F, 2 * F, 3 * F

            def mm_pass(lhs_tiles, rhs_get, v, c):
                ps = psp.tile([128, 512], fp, tag="ps", name="ps")
                for k in range(NT):
                    nc.tensor.matmul(
                        ps[:rows[v], :CH[c]],
                        lhs_tiles[k][:rows[k], off[v]:off[v] + rows[v]],
                        rhs_get(k)[:rows[k], :],
                        start=(k == 0), stop=(k == NT - 1))
                return ps

            # pass 1 (dir1): s1 = 0.5*z + m1 ; (dir2): s2 = dinv*(z'+q2) + q1
            for v in range(NT):
                for c in range(3):
                    sl = slice(coff[c], coff[c] + CH[c])
                    ps = mm_pass(aT, lambda k: pn[k][:, P2 + coff[c]:P2 + coff[c] + CH[c]], v, c)
                    nc.vector.scalar_tensor_tensor(
                        s1[v][:rows[v], sl], ps[:rows[v], :CH[c]], 0.5,
                        pn[v][:rows[v], M1 + coff[c]:M1 + coff[c] + CH[c]], op0=MUL, op1=ADD)
                    ps2 = mm_pass(aD, lambda k: pn[k][:, Q2 + coff[c]:Q2 + coff[c] + CH[c]], v, c)
                    t = wk.tile([128, 512], fp, tag="t")
                    nc.vector.tensor_add(t[:rows[v], :CH[c]], ps2[:rows[v], :CH[c]],
                                         pn[v][:rows[v], Q2 + coff[c]:Q2 + coff[c] + CH[c]])
                    nc.vector.scalar_tensor_tensor(
                        s2[v][:rows[v], sl], t[:rows[v], :CH[c]], dv[:rows[v], v:v + 1],
                        pn[v][:rows[v], Q1 + coff[c]:Q1 + coff[c] + CH[c]], op0=MUL, op1=ADD)

            # pass 2: u = p0 + 0.5*(z1+s1) + dinv*(z1'+s2)
            for v in range(NT):
                p0t = wk.tile([128, F], fp, tag="p0")
                nc.sync.dma_start(out=p0t[:rows[v], :], in_=p0_d[off[v]:off[v] + rows[v], :])
                for c in range(3):
                    sl = slice(coff[c], coff[c] + CH[c])
                    ps = mm_pass(aT, lambda k: s1[k][:rows[k], sl], v, c)
                    w1 = wk.tile([128, 512], fp, tag="w1")
                    nc.vector.tensor_add(w1[:rows[v], :CH[c]], ps[:rows[v], :CH[c]],
                                         s1[v][:rows[v], sl])
                    ut = wk.tile([128, 512], fp, tag="ut")
                    nc.vector.scalar_tensor_tensor(
                        ut[:rows[v], :CH[c]], w1[:rows[v], :CH[c]], 0.5,
                        p0t[:rows[v], sl], op0=MUL, op1=ADD)
                    ps2 = mm_pass(aD, lambda k: s2[k][:rows[k], sl], v, c)
                    w2 = wk.tile([128, 512], fp, tag="w2")
                    nc.vector.tensor_add(w2[:rows[v], :CH[c]], ps2[:rows[v], :CH[c]],
                                         s2[v][:rows[v], sl])
                    uo = wk.tile([128, 512], fp, tag="uo")
                    nc.vector.scalar_tensor_tensor(
                        uo[:rows[v], :CH[c]], w2[:rows[v], :CH[c]], dv[:rows[v], v:v + 1],
                        ut[:rows[v], :CH[c]], op0=MUL, op1=ADD)
                    nc.sync.dma_start(out=u_d[off[v]:off[v] + rows[v], sl],
                                      in_=uo[:rows[v], :CH[c]])
    return nc


def _device_apply(adp, dinv, p0, p1, p2, q1, q2):
    """p*: [B, 8, N, t] (t <= TP). Returns u [B, 8, N, t] or None on failure."""
    if _DEV["fail"]:
        return None
    try:
        from concourse.bass_utils import run_bass_kernel_spmd
        import ml_dtypes
        if _DEV["nc"] is None:
            _DEV["nc"] = _build_nc()
        nc = _DEV["nc"]
        t = p0.shape[-1]
        bf16 = ml_dtypes.bfloat16

        def padpack(x):  # [8,N,t] -> [N, F]
            o = np.zeros((RC, N, TP), f32)
            o[:, :, :t] = x
            return o.transpose(1, 0, 2).reshape(N, F)

        dpad = np.zeros((1024,), f32)
        dpad[:N] = dinv
        dmat = dpad.reshape(8, 128).T.copy()
        in_maps = []
        for b in range(B):
            pnb = np.concatenate(
                [padpack(p2[b]), padpack(p1[b] + 0.5 * p2[b]),
                 padpack(q2[b]), padpack(q1[b])], axis=1).astype(bf16)
            in_maps.append({
                "pn": pnb,
                "p0": padpack(p0[b]).astype(f32),
                "adpT": adp.T.astype(bf16).copy(),
                "adp": adp.astype(bf16).copy(),
                "dinv": dmat.astype(f32),
            })
        res = run_bass_kernel_spmd(nc, in_maps, list(range(B)))
        outs = []
        for b in range(B):
            ub = np.asarray(res.results[b]["u"], f32).reshape(N, RC, TP)
            outs.append(ub.transpose(1, 0, 2)[:, :, :t])
        return np.stack(outs, 0)
    except Exception as e:  # fall back to exact host math
        import traceback
        traceback.print_exc()
        _DEV["fail"] = True
        return None


# ---------------- full forward ----------------
def kernel(**d):
    d = {k: np.asarray(v) for k, v in d.items()}
    adp = _graph_prep(d)
    dinv = (1.0 / (1.0 + adp.sum(axis=0))).astype(f32)
    x = np.einsum("bint,oi->bont", d["input"], d["start_w"]).astype(f32) + \
        d["start_b"][None, :, None, None]
    for l in range(L):
        T = x.shape[-1]
        Tp = T - 7
        filts, gates = [], []
        for k in KSET:
            w, bias = d["fw%d" % k][l], d["fb%d" % k][l]
            kk = w.shape[-1]
            acc = np.zeros((B, w.shape[0], N, T - kk + 1), f32)
            for j in range(kk):
                acc += np.einsum("oi,bint->bont", w[:, :, 0, j],
                                 x[:, :, :, j:T - kk + 1 + j])
            filts.append((acc + bias[None, :, None, None])[..., -Tp:])
            w, bias = d["gw%d" % k][l], d["gb%d" % k][l]
            acc = np.zeros((B, w.shape[0], N, T - kk + 1), f32)
            for j in range(kk):
                acc += np.einsum("oi,bint->bont", w[:, :, 0, j],
                                 x[:, :, :, j:T - kk + 1 + j])
            gates.append((acc + bias[None, :, None, None])[..., -Tp:])
        filt = np.tanh(np.concatenate(filts, 1))
        gate = 1.0 / (1.0 + np.exp(-np.concatenate(gates, 1)))
        x1 = (filt * gate).astype(f32)                      # [B,32,N,Tp]
        Q0, Q1, Q2, R0, R1, R2, ub = _fold(d, l)
        p0 = np.einsum("oc,bcnt->bont", Q0 + R0, x1).astype(f32)
        p1 = np.einsum("oc,bcnt->bont", Q1, x1).astype(f32)
        p2 = np.einsum("oc,bcnt->bont", Q2, x1).astype(f32)
        q1 = np.einsum("oc,bcnt->bont", R1, x1).astype(f32)
        q2 = np.einsum("oc,bcnt->bont", R2, x1).astype(f32)
        u = _device_apply(adp, dinv, p0, p1, p2, q1, q2)
        if u is None:
            u = _host_apply(adp, dinv, p0, p1, p2, q1, q2)
        u = u + ub[None, :, None, None].astype(f32) + x[:, :, :, -Tp:]
        mu = u.mean(axis=(1, 2, 3), keepdims=True)
        var = u.var(axis=(1, 2, 3), keepdims=True)
        x = ((u - mu) / np.sqrt(var + EPS)).astype(f32)
    T = x.shape[-1]
    p = np.zeros((TSHORT, T), f32)
    for i in range(TSHORT):
        s = (i * T) // TSHORT
        e = -((-(i + 1) * T) // TSHORT)
        p[i, s:e] = 1.0 / (e - s)
    return np.einsum("st,bcnt->bcsn", p, x).astype(f32)



# revision 7
# speedup vs baseline: 1.2650x; 1.2650x over previous
"""Trainium2 Bass kernel for nn_LongTermEncoder (gnn_message_passing).

Sharding: data-parallel over batch B=8 across 8 NeuronCores (adjacency and
all params replicated).  The ENTIRE forward runs on-device in ONE compiled
kernel per core: inception convs (block-diagonal window matmuls), tanh/
sigmoid gating, channel projections (with a ones-row carrying the mixprop
bias), dense adjacency message passing (both directions, 2 hops, folded
through the channel projections exactly as in the reference), residual,
per-sample layernorm, and adaptive average pooling.  The host only builds
the dynamic adjacency (small numpy), folds weights, and reassembles output.

mixprop refactor (exact):  out = Q0 x + A(Q1 x + A(Q2 x)) + B(R1 x + B(R2 x)),
A=(adp+I)/2, B = D^-1(adp^T+I); channel mixing commutes with node mixing.

This container's walrus build rejects instructions with more than ~1 inline
semaphore wait ("Too many sync wait commands"), which Tile emits freely —
every nontrivial Tile kernel fails codegen.  We post-process the BIR JSON
(hoisting excess waits onto EventSemaphore carrier instructions on the same
engine queue, which preserves semantics) via a monkeypatch around
compile_bir_kernel.
"""
import os
import numpy as np

L, GDEP, PA, ALPHA, KTOP, TSHORT, EPS = 3, 2, 0.05, 3.0, 20, 12, 1e-5
KSET = (2, 4, 6, 8)
N, B, RC, CC = 1000, 8, 8, 32
f32 = np.float32

# per-layer time extents
TIN = (168, 161, 154)
TP = (161, 154, 147)
TPMAX = 161

_CACHE_DIR = os.environ.get("JAX_COMPILATION_CACHE_DIR", "/tmp/jaxcache_lte")


# ---------------------------------------------------------------- BIR fix
_LIMITS = {"NoOp": 0, "Drain": 0}
_EV_LIM = 1


def _fix_bir_waits(bir_bytes):
    import orjson
    d = orjson.loads(bir_bytes)
    ctr = 0
    changed = False
    for fn in d.get("functions", []):
        for blk in fn.get("blocks", []):
            newl = []
            for ins in blk.get("instructions", []):
                si = ins.get("sync_info")
                ow = (si or {}).get("on_wait") or []
                lim = _LIMITS.get(ins.get("opcode"), 1)
                if len(ow) > lim:
                    changed = True
                    regw = [w for w in ow if w.get("wait_reg") is not None]
                    immw = [w for w in ow if w.get("wait_reg") is None]
                    keep = (regw + immw)[:lim]
                    excess = (regw + immw)[lim:]
                    ins["sync_info"]["on_wait"] = keep
                    for i in range(0, len(excess), _EV_LIM):
                        ctr += 1
                        newl.append({
                            "debug": ins.get("debug", 0),
                            "engine": ins["engine"],
                            "ins": [], "outs": [],
                            "name": "wsplit-%d" % ctr,
                            "opcode": "EventSemaphore",
                            "sync_info": {"on_update": [],
                                          "on_wait": excess[i:i + _EV_LIM]},
                        })
                newl.append(ins)
            blk["instructions"] = newl
    return orjson.dumps(d) if changed else bir_bytes


_patched = [False]


def _install_birfix():
    if _patched[0]:
        return
    _patched[0] = True
    from concourse import bass2jax, bass_utils
    orig = bass_utils.compile_bir_kernel

    def patched(bir_json, tmpdir, neff_name="file.neff"):
        return orig(_fix_bir_waits(bytes(bir_json)), tmpdir, neff_name)

    bass2jax.compile_bir_kernel = patched


# ---------------------------------------------------------------- host math
def _graph_prep(d):
    emb1, emb2 = d["emb1"], d["emb2"]
    v1 = np.tanh(ALPHA * (emb1 @ d["lin1_w"].T + d["lin1_b"])).astype(f32)
    v2 = np.tanh(ALPHA * (emb2 @ d["lin2_w"].T + d["lin2_b"])).astype(f32)
    a = v1 @ v2.T - v2 @ v1.T
    adj = np.maximum(np.tanh(ALPHA * a), 0.0).astype(f32)
    score = adj + f32(0.01) * d["topk_noise"]
    t1 = np.argpartition(-score, KTOP, axis=1)[:, :KTOP]
    mask = np.zeros((N, N), f32)
    np.put_along_axis(mask, t1, 1.0, axis=1)
    adp = adj * mask
    mv = (1.0 - d["cooldowns"]).astype(f32)
    z = adp * (mv[:, None] * mv[None, :])
    z = z - z.max(axis=1, keepdims=True)
    e = np.exp(z)
    return (e / e.sum(axis=1, keepdims=True)).astype(f32)


def _fold(d, l):
    W = d["g1_w"][l]
    W0, W1, W2 = W[:, :32], W[:, 32:64], W[:, 64:]
    V = d["g2_w"][l]
    V0, V1, V2 = V[:, :32], V[:, 32:64], V[:, 64:]
    al, g = PA, 1.0 - PA
    Q0 = W0 + al * W1 + al * W2
    Q1 = g * W1 + g * al * W2
    Q2 = g * g * W2
    R0 = V0 + al * V1 + al * V2
    R1 = g * V1 + g * al * V2
    R2 = g * g * V2
    ub = d["g1_b"][l] + d["g2_b"][l]
    return Q0, Q1, Q2, R0, R1, R2, ub


# my x1 row order [k8,k6,k4,k2] -> reference channel index (KSET concat order)
_PERM = np.array([24 + i for i in range(8)] + [16 + i for i in range(8)] +
                 [8 + i for i in range(8)] + [0 + i for i in range(8)])


def _prep_weights(d):
    """Per-layer device weights: WA [112,32], WB [48,32], W33 [33,40],
    bias tiles bA/bB [32,1]."""
    import ml_dtypes
    bf16 = ml_dtypes.bfloat16
    out = {}
    for l in range(L):
        WA = np.zeros((112, 48), f32)
        WB = np.zeros((48, 48), f32)
        fw8, fw6 = d["fw8"][l], d["fw6"][l]
        gw8, gw6 = d["gw8"][l], d["gw6"][l]
        fw4, fw2 = d["fw4"][l], d["fw2"][l]
        gw4, gw2 = d["gw4"][l], d["gw2"][l]
        for tap in range(8):
            for ch in range(8):
                r = tap * 8 + ch
                WA[r, 0:8] = fw8[:, ch, 0, tap]
                WA[r, 32:40] = gw8[:, ch, 0, tap]
        for tap in range(2, 8):
            for ch in range(8):
                r = 64 + (tap - 2) * 8 + ch
                WA[r, 8:16] = fw6[:, ch, 0, tap - 2]
                WA[r, 40:48] = gw6[:, ch, 0, tap - 2]
        for tap in range(4, 8):
            for ch in range(8):
                r = (tap - 4) * 8 + ch
                WB[r, 0:8] = fw4[:, ch, 0, tap - 4]
                WB[r, 32:40] = gw4[:, ch, 0, tap - 4]
        for tap in range(6, 8):
            for ch in range(8):
                r = 32 + (tap - 6) * 8 + ch
                WB[r, 8:16] = fw2[:, ch, 0, tap - 6]
                WB[r, 40:48] = gw2[:, ch, 0, tap - 6]
        z8 = np.zeros((16,), f32)
        bA = np.concatenate([d["fb8"][l], d["fb6"][l], z8,
                             d["gb8"][l], d["gb6"][l]]).astype(f32)[:, None]
        bB = np.concatenate([d["fb4"][l], d["fb2"][l], z8,
                             d["gb4"][l], d["gb2"][l]]).astype(f32)[:, None]
        Q0, Q1, Q2, R0, R1, R2, ub = _fold(d, l)
        W33 = np.zeros((65, 40), f32)
        blocks = [Q2, Q1 + 0.5 * Q2, R2, R1, Q0 + R0]
        for bi, Qx in enumerate(blocks):
            qp = Qx[:, _PERM].T          # [32, 8] rows in (k8,k6,k4,k2) order
            W33[0:16, bi * 8:(bi + 1) * 8] = qp[0:16]
            W33[32:48, bi * 8:(bi + 1) * 8] = qp[16:32]
        W33[64, 32:40] = ub
        out["wa%d" % l] = WA.astype(bf16)
        out["wb%d" % l] = WB.astype(bf16)
        out["w33_%d" % l] = W33.astype(bf16)
        out["ba%d" % l] = bA
        out["bb%d" % l] = bB
    return out


def _pool_bounds(t_in):
    bnds = []
    for s in range(TSHORT):
        a = (s * t_in) // TSHORT
        b = -((-(s + 1) * t_in) // TSHORT)
        bnds.append((a, b - a))
    return bnds


# ---------------------------------------------------------------- device
_DEV = {"nc": None, "fail": False}


def _build_nc():
    import concourse.bass as bass
    import concourse.mybir as mybir
    from concourse.tile import TileContext

    bf = mybir.dt.bfloat16
    fp = mybir.dt.float32
    AF = mybir.ActivationFunctionType
    MUL = mybir.AluOpType.mult
    ADD = mybir.AluOpType.add
    SUB = mybir.AluOpType.subtract
    AX = mybir.AxisListType.X

    nc = bass.Bass()
    x0_d = nc.declare_dram_parameter("x0", (N, 8 * TIN[0]), fp, isOutput=False)
    adpT_d = nc.declare_dram_parameter("adpT", (N, N), bf, isOutput=False)
    adp_d = nc.declare_dram_parameter("adp", (N, N), bf, isOutput=False)
    dinv_d = nc.declare_dram_parameter("dinv", (128, 8), fp, isOutput=False)
    wparams = {}
    for l in range(L):
        wparams["wa%d" % l] = nc.declare_dram_parameter("wa%d" % l, (112, 48), bf, isOutput=False)
        wparams["wb%d" % l] = nc.declare_dram_parameter("wb%d" % l, (48, 48), bf, isOutput=False)
        wparams["w33_%d" % l] = nc.declare_dram_parameter("w33_%d" % l, (65, 40), bf, isOutput=False)
        wparams["ba%d" % l] = nc.declare_dram_parameter("ba%d" % l, (48, 1), fp, isOutput=False)
        wparams["bb%d" % l] = nc.declare_dram_parameter("bb%d" % l, (48, 1), fp, isOutput=False)
    out_d = nc.declare_dram_parameter("out", (N, 8 * TSHORT), fp, isOutput=True)

    NT = 8
    rows = [128] * 7 + [104]
    off = [128 * i for i in range(NT)]
    NCH = 64  # conv node-chunk

    with TileContext(nc) as tc:
        with tc.tile_pool(name="glob", bufs=1) as glob, \
             tc.tile_pool(name="dram", bufs=1, space="DRAM") as drp:
            # resident: adjacency, dinv, weights, ones helpers
            aT, aD = [], []
            for k in range(NT):
                t = glob.tile([128, N], bf, tag="aT%d" % k)
                nc.sync.dma_start(out=t[:rows[k], :], in_=adpT_d[off[k]:off[k] + rows[k], :])
                aT.append(t)
                t = glob.tile([128, N], bf, tag="aD%d" % k)
                nc.sync.dma_start(out=t[:rows[k], :], in_=adp_d[off[k]:off[k] + rows[k], :])
                aD.append(t)
            dv = glob.tile([128, 8], fp, tag="dinv")
            nc.sync.dma_start(out=dv[:, :], in_=dinv_d[:, :])
            wt = {}
            for l in range(L):
                for nm, shp in (("wa%d" % l, (112, 48)), ("wb%d" % l, (48, 48)),
                                ("w33_%d" % l, (65, 40))):
                    t = glob.tile([shp[0], shp[1]], bf, tag=nm)
                    nc.sync.dma_start(out=t[:, :], in_=wparams[nm][:, :])
                    wt[nm] = t
                for nm in ("ba%d" % l, "bb%d" % l):
                    t = glob.tile([48, 1], fp, tag=nm)
                    nc.sync.dma_start(out=t[:, :], in_=wparams[nm][:, :])
                    wt[nm] = t
            onescol = glob.tile([128, 1], fp, tag="onescol")
            nc.vector.memset(onescol[:, :], 1.0)
            onesrow = glob.tile([1, 128], fp, tag="onesrow")
            nc.vector.memset(onesrow[:, :], 1.0)
            bc = glob.tile([128, 2], fp, tag="bc")  # (negmu, inv) broadcast

            # DRAM scratch
            pall_s = drp.tile([40, N * TPMAX], bf, tag="pall", name="pall_s")
            s1_s = drp.tile([8, N * TPMAX], bf, tag="s1", name="s1_s")
            s2_s = drp.tile([8, N * TPMAX], bf, tag="s2", name="s2_s")
            xn = [None,
                  drp.tile([N, 8 * TP[0]], fp, tag="xn1", name="xn1"),
                  drp.tile([N, 8 * TP[1]], fp, tag="xn2", name="xn2"),
                  drp.tile([N, 8 * TP[2]], fp, tag="xn3", name="xn3")]

            for l in range(L):
                T, Tp = TIN[l], TP[l]
                xin = x0_d if l == 0 else xn[l]
                F = 8 * Tp

                # ---------------- conv + gating + projection ----------------
                with tc.tile_pool(name="cv%d" % l, bufs=1) as cvp, \
                     tc.tile_pool(name="cvps%d" % l, bufs=1, space="PSUM") as cps:
                    for n0 in range(0, N, NCH):
                        nn = min(NCH, N - n0)
                        cols = nn * Tp
                        xwA = cvp.tile([112, NCH * TPMAX], bf, tag="xwA", name="xwA")
                        xwB = cvp.tile([48, NCH * TPMAX], bf, tag="xwB", name="xwB")
                        xsrc = xin[n0:n0 + nn, :].rearrange("n (c t) -> c n t", t=T)
                        # window loads (cast f32->bf16 via gpsimd)
                        for tap in range(8):
                            nc.gpsimd.dma_start(
                                out=xwA[tap * 8:tap * 8 + 8, :cols].rearrange("r (n t) -> r n t", t=Tp),
                                in_=xsrc[:, :, tap:tap + Tp])
                        for tap in range(2, 8):
                            r = 64 + (tap - 2) * 8
                            nc.gpsimd.dma_start(
                                out=xwA[r:r + 8, :cols].rearrange("r (n t) -> r n t", t=Tp),
                                in_=xsrc[:, :, tap:tap + Tp])
                        for tap in range(4, 8):
                            r = (tap - 4) * 8
                            nc.gpsimd.dma_start(
                                out=xwB[r:r + 8, :cols].rearrange("r (n t) -> r n t", t=Tp),
                                in_=xsrc[:, :, tap:tap + Tp])
                        for tap in range(6, 8):
                            r = 32 + (tap - 6) * 8
                            nc.gpsimd.dma_start(
                                out=xwB[r:r + 8, :cols].rearrange("r (n t) -> r n t", t=Tp),
                                in_=xsrc[:, :, tap:tap + Tp])

                        fsb = cvp.tile([65, NCH * TPMAX], bf, tag="fsb", name="fsb")
                        gsb = cvp.tile([48, NCH * TPMAX], bf, tag="gsb", name="gsb")
                        nc.vector.memset(fsb[:, :cols], 0.0)
                        nc.vector.memset(fsb[64:65, :cols], 1.0)
                        wa, wb = wt["wa%d" % l], wt["wb%d" % l]
                        ba, bb = wt["ba%d" % l], wt["bb%d" % l]
                        for sp0 in range(0, cols, 2048):
                            sw = min(2048, cols - sp0)
                            psA = cps.tile([48, 2048], fp, tag="psA", name="psA")
                            psB = cps.tile([48, 2048], fp, tag="psB", name="psB")
                            for c0 in range(0, sw, 512):
                                cw = min(512, sw - c0)
                                nc.tensor.matmul(psA[:48, c0:c0 + cw], wa[:, :],
                                                 xwA[:, sp0 + c0:sp0 + c0 + cw],
                                                 start=True, stop=True)
                                nc.tensor.matmul(psB[:48, c0:c0 + cw], wb[:, :],
                                                 xwB[:, sp0 + c0:sp0 + c0 + cw],
                                                 start=True, stop=True)
                            sl = slice(sp0, sp0 + sw)
                            nc.scalar.activation(fsb[0:16, sl], psA[0:16, :sw], AF.Tanh, bias=ba[0:16, 0:1])
                            nc.scalar.activation(fsb[32:48, sl], psB[0:16, :sw], AF.Tanh, bias=bb[0:16, 0:1])
                            nc.scalar.activation(gsb[0:16, sl], psA[32:48, :sw], AF.Sigmoid, bias=ba[32:48, 0:1])
                            nc.scalar.activation(gsb[32:48, sl], psB[32:48, :sw], AF.Sigmoid, bias=bb[32:48, 0:1])
                        nc.vector.tensor_mul(fsb[0:16, :cols], fsb[0:16, :cols], gsb[0:16, :cols])
                        nc.vector.tensor_mul(fsb[32:48, :cols], fsb[32:48, :cols], gsb[32:48, :cols])
                        # projection to 40 rows + dump to DRAM
                        w33 = wt["w33_%d" % l]
                        for sp0 in range(0, cols, 2048):
                            sw = min(2048, cols - sp0)
                            psP = cps.tile([40, 2048], fp, tag="psA", name="psP")
                            for c0 in range(0, sw, 512):
                                cw = min(512, sw - c0)
                                nc.tensor.matmul(psP[:40, c0:c0 + cw], w33[:, :],
                                                 fsb[:, sp0 + c0:sp0 + c0 + cw],
                                                 start=True, stop=True)
                            stg = cvp.tile([40, 2048], bf, tag="stg", name="stg")
                            nc.scalar.activation(stg[:, :sw], psP[:, :sw], AF.Copy)
                            nc.sync.dma_start(
                                out=pall_s[:, n0 * Tp + sp0:n0 * Tp + sp0 + sw],
                                in_=stg[:, :sw])

                # ---------------- adjacency passes ----------------
                def pall_blk(b0, k):  # rhs [rows_k, 8*Tp] bf16 view of block b0
                    return pall_s[b0 * 8:b0 * 8 + 8,
                                  off[k] * Tp:(off[k] + rows[k]) * Tp].rearrange(
                                      "c (n t) -> n c t", t=Tp)

                def s_blk(s_s, k):
                    return s_s[:, off[k] * Tp:(off[k] + rows[k]) * Tp].rearrange(
                        "c (n t) -> n c t", t=Tp)

                with tc.tile_pool(name="ad%d" % l, bufs=1) as adp_, \
                     tc.tile_pool(name="adw%d" % l, bufs=3) as adw, \
                     tc.tile_pool(name="adps%d" % l, bufs=1, space="PSUM") as aps:

                    def sweep(lhs, rhs_get, v, ps):
                        for k in range(NT):
                            rt = adw.tile([128, F], bf, tag="rt", name="rt")
                            nc.sync.dma_start(
                                out=rt[:rows[k], :].rearrange("n (c t) -> n c t", t=Tp),
                                in_=rhs_get(k))
                            for c0 in range(0, F, 512):
                                cw = min(512, F - c0)
                                nc.tensor.matmul(ps[:rows[v], c0:c0 + cw],
                                                 lhs[k][:rows[k], off[v]:off[v] + rows[v]],
                                                 rt[:rows[k], c0:c0 + cw],
                                                 start=(k == 0), stop=(k == NT - 1))

                    # pass 1: s1 = 0.5 z + m1 ; s2 = dinv (z' + q2) + q1
                    for v in range(NT):
                        ps = aps.tile([128, F], fp, tag="psZ", name="psZ1")
                        sweep(aT, lambda k: pall_blk(0, k), v, ps)
                        m1v = adw.tile([128, F], fp, tag="in1", name="m1v")
                        nc.gpsimd.dma_start(
                            out=m1v[:rows[v], :].rearrange("n (c t) -> n c t", t=Tp),
                            in_=pall_blk(1, v))
                        s1o = adw.tile([128, F], bf, tag="so", name="s1o")
                        nc.vector.scalar_tensor_tensor(
                            s1o[:rows[v], :], ps[:rows[v], :], 0.5,
                            m1v[:rows[v], :], op0=MUL, op1=ADD)
                        nc.sync.dma_start(out=s_blk(s1_s, v), in_=s1o[:rows[v], :].rearrange("n (c t) -> n c t", t=Tp))

                        ps2 = aps.tile([128, F], fp, tag="psZ", name="psZ2")
                        sweep(aD, lambda k: pall_blk(2, k), v, ps2)
                        q2v = adw.tile([128, F], fp, tag="in1", name="q2v")
                        nc.gpsimd.dma_start(
                            out=q2v[:rows[v], :].rearrange("n (c t) -> n c t", t=Tp),
                            in_=pall_blk(2, v))
                        q1v = adw.tile([128, F], fp, tag="in1", name="q1v")
                        nc.gpsimd.dma_start(
                            out=q1v[:rows[v], :].rearrange("n (c t) -> n c t", t=Tp),
                            in_=pall_blk(3, v))
                        w2 = adw.tile([128, F], fp, tag="w2", name="w2")
                        nc.vector.tensor_add(w2[:rows[v], :], ps2[:rows[v], :], q2v[:rows[v], :])
                        s2o = adw.tile([128, F], bf, tag="so", name="s2o")
                        nc.vector.scalar_tensor_tensor(
                            s2o[:rows[v], :], w2[:rows[v], :], dv[:rows[v], v:v + 1],
                            q1v[:rows[v], :], op0=MUL, op1=ADD)
                        nc.sync.dma_start(out=s_blk(s2_s, v), in_=s2o[:rows[v], :].rearrange("n (c t) -> n c t", t=Tp))

                    # pass 2: u = p0 + 0.5 (z1 + s1) + dinv (z2 + s2) + resid
                    uts = []
                    stats = adp_.tile([128, 16], fp, tag="stats")
                    nc.vector.memset(stats[:, :], 0.0)
                    for v in range(NT):
                        ps = aps.tile([128, F], fp, tag="psZ", name="psZ3")
                        sweep(aT, lambda k: s_blk(s1_s, k), v, ps)
                        s1v = adw.tile([128, F], fp, tag="in1", name="s1v")
                        nc.gpsimd.dma_start(out=s1v[:rows[v], :].rearrange("n (c t) -> n c t", t=Tp), in_=s_blk(s1_s, v))
                        p0v = adw.tile([128, F], fp, tag="in1", name="p0v")
                        nc.gpsimd.dma_start(out=p0v[:rows[v], :].rearrange("n (c t) -> n c t", t=Tp), in_=pall_blk(4, v))
                        w1 = adw.tile([128, F], fp, tag="w2", name="w1b")
                        nc.vector.tensor_add(w1[:rows[v], :], ps[:rows[v], :], s1v[:rows[v], :])
                        ut = adp_.tile([128, F], fp, tag="ut%d" % v, name="ut%d" % v)
                        nc.vector.scalar_tensor_tensor(
                            ut[:rows[v], :], w1[:rows[v], :], 0.5,
                            p0v[:rows[v], :], op0=MUL, op1=ADD)

                        ps2 = aps.tile([128, F], fp, tag="psZ", name="psZ4")
                        sweep(aD, lambda k: s_blk(s2_s, k), v, ps2)
                        s2v = adw.tile([128, F], fp, tag="in1", name="s2v")
                        nc.gpsimd.dma_start(out=s2v[:rows[v], :].rearrange("n (c t) -> n c t", t=Tp), in_=s_blk(s2_s, v))
                        w2 = adw.tile([128, F], fp, tag="w2", name="w2b")
                        nc.vector.tensor_add(w2[:rows[v], :], ps2[:rows[v], :], s2v[:rows[v], :])
                        nc.vector.scalar_tensor_tensor(
                            ut[:rows[v], :], w2[:rows[v], :], dv[:rows[v], v:v + 1],
                            ut[:rows[v], :], op0=MUL, op1=ADD)
                        # residual (last Tp of each channel of x_in)
                        xr = adw.tile([128, F], fp, tag="xr", name="xr")
                        nc.sync.dma_start(
                            out=xr[:rows[v], :].rearrange("n (c t) -> n c t", t=Tp),
                            in_=xin[off[v]:off[v] + rows[v], :].rearrange(
                                "n (c t) -> n c t", t=T)[:, :, T - Tp:])
                        nc.vector.tensor_add(ut[:rows[v], :], ut[:rows[v], :], xr[:rows[v], :])
                        # stats partials
                        nc.vector.reduce_sum(stats[:rows[v], 2 * v:2 * v + 1], ut[:rows[v], :], axis=AX)
                        usq = adw.tile([128, F], fp, tag="w2", name="usq")
                        nc.vector.tensor_mul(usq[:rows[v], :], ut[:rows[v], :], ut[:rows[v], :])
                        nc.vector.reduce_sum(stats[:rows[v], 2 * v + 1:2 * v + 2], usq[:rows[v], :], axis=AX)
                        uts.append(ut)

                    # layernorm stats across partitions
                    psS = aps.tile([1, 16], fp, tag="psS", name="psS")
                    nc.tensor.matmul(psS[:1, :16], onescol[:, :], stats[:, :], start=True, stop=True)
                    CNT = float(8 * N * Tp)
                    sm = adp_.tile([1, 8], fp, tag="sm")
                    nc.vector.reduce_sum(sm[:1, 0:1], psS[0:1, :].rearrange("p (v two) -> p two v", two=2)[:, 0:1, :], axis=AX)
                    nc.vector.reduce_sum(sm[:1, 1:2], psS[0:1, :].rearrange("p (v two) -> p two v", two=2)[:, 1:2, :], axis=AX)
                    # mu = sm0/CNT ; var = sm1/CNT - mu^2 ; inv = 1/sqrt(var+EPS)
                    nc.vector.tensor_scalar_mul(sm[:1, 2:3], sm[:1, 0:1], 1.0 / CNT)   # mu
                    nc.vector.tensor_scalar_mul(sm[:1, 3:4], sm[:1, 1:2], 1.0 / CNT)   # E[x^2]
                    nc.vector.tensor_mul(sm[:1, 4:5], sm[:1, 2:3], sm[:1, 2:3])        # mu^2
                    nc.vector.tensor_sub(sm[:1, 5:6], sm[:1, 3:4], sm[:1, 4:5])        # var
                    nc.vector.tensor_scalar_add(sm[:1, 5:6], sm[:1, 5:6], float(EPS))
                    nc.scalar.activation(sm[:1, 6:7], sm[:1, 5:6], AF.Sqrt)
                    nc.vector.reciprocal(sm[:1, 7:8], sm[:1, 6:7])                     # inv
                    nc.vector.tensor_scalar_mul(sm[:1, 2:3], sm[:1, 2:3], -1.0)        # -mu
                    bc2 = adp_.tile([1, 2], fp, tag="bc2")
                    nc.vector.tensor_copy(bc2[:1, 0:1], sm[:1, 2:3])
                    nc.vector.tensor_copy(bc2[:1, 1:2], sm[:1, 7:8])
                    psb = aps.tile([128, 2], fp, tag="psS", name="psb")
                    nc.tensor.matmul(psb[:, :2], onesrow[:, :], bc2[:1, :2], start=True, stop=True)
                    nc.scalar.activation(bc[:, :2], psb[:, :2], AF.Copy)

                    # apply LN, write x_{l+1} (or pooling input) to DRAM
                    for v in range(NT):
                        xo = adw.tile([128, F], fp, tag="xo", name="xo")
                        nc.vector.tensor_scalar(
                            xo[:rows[v], :], uts[v][:rows[v], :],
                            bc[:rows[v], 0:1], bc[:rows[v], 1:2], op0=ADD, op1=MUL)
                        if l < L - 1:
                            nc.sync.dma_start(out=xn[l + 1][off[v]:off[v] + rows[v], :], in_=xo[:rows[v], :])
                        else:
                            nc.sync.dma_start(out=xn[3][off[v]:off[v] + rows[v], :], in_=xo[:rows[v], :])

            # ---------------- adaptive avg pooling ----------------
            with tc.tile_pool(name="pool", bufs=2) as plp:
                bnds = _pool_bounds(TP[2])
                for v in range(NT):
                    xt = plp.tile([128, 8 * TP[2]], fp, tag="xt", name="xt")
                    nc.sync.dma_start(out=xt[:rows[v], :], in_=xn[3][off[v]:off[v] + rows[v], :])
                    ot = plp.tile([128, 8 * TSHORT], fp, tag="ot", name="ot")
                    xv = xt[:rows[v], :].rearrange("n (c t) -> n c t", t=TP[2])
                    for s, (a, w) in enumerate(bnds):
                        nc.vector.reduce_sum(ot[:rows[v], s * 8:(s + 1) * 8],
                                             xv[:, :, a:a + w], axis=AX)
                    nc.sync.dma_start(out=out_d[off[v]:off[v] + rows[v], :], in_=ot[:rows[v], :8 * TSHORT])
    return nc


def _device_forward(d, adp):
    try:
        import jax
        try:
            jax.config.update("jax_compilation_cache_dir", _CACHE_DIR)
            jax.config.update("jax_persistent_cache_min_compile_time_secs", 0.0)
            jax.config.update("jax_persistent_cache_min_entry_size_bytes", -1)
        except Exception:
            pass
        _install_birfix()
        from concourse.bass_utils import run_bass_kernel_spmd
        import ml_dtypes
        bf16 = ml_dtypes.bfloat16

        dinv = (1.0 / (1.0 + adp.sum(axis=0))).astype(f32)
        dpad = np.zeros((1024,), f32)
        dpad[:N] = dinv
        dmat = np.ascontiguousarray(dpad.reshape(8, 128).T)

        # start conv on host: x0 [B, N, 8, T]
        inp = d["input"]
        xf = inp.transpose(0, 2, 3, 1).reshape(-1, 2) @ d["start_w"].T
        xf += d["start_b"][None, :]
        x0 = np.ascontiguousarray(
            xf.reshape(B, N, TIN[0], 8).transpose(0, 1, 3, 2)).reshape(B, N, 8 * TIN[0])

        wts = _prep_weights(d)
        adpT_b = np.ascontiguousarray(adp.T).astype(bf16)
        adp_b = adp.astype(bf16)
        if _DEV["nc"] is None:
            _DEV["nc"] = _build_nc()
        in_maps = []
        for b in range(B):
            m = {"x0": x0[b].astype(f32), "adpT": adpT_b, "adp": adp_b,
                 "dinv": dmat}
            m.update(wts)
            in_maps.append(m)
        res = run_bass_kernel_spmd(_DEV["nc"], in_maps, list(range(B)))
        scale = np.array([1.0 / w for (_, w) in _pool_bounds(TP[2])], f32)
        outs = []
        for b in range(B):
            ob = np.asarray(res.results[b]["out"], f32).reshape(N, TSHORT, 8)
            outs.append(ob.transpose(2, 1, 0) * scale[None, :, None])
        return np.stack(outs, 0)
    except Exception:
        import traceback
        traceback.print_exc()
        _DEV["fail"] = True
        return None


# ---------------------------------------------------------------- host fallback
def _host_forward(d, adp):
    dinv = (1.0 / (1.0 + adp.sum(axis=0))).astype(f32)
    x = np.einsum("bint,oi->bont", d["input"], d["start_w"]).astype(f32) + \
        d["start_b"][None, :, None, None]
    for l in range(L):
        T = x.shape[-1]
        Tp = T - 7
        filts, gates = [], []
        for k in KSET:
            w, bias = d["fw%d" % k][l], d["fb%d" % k][l]
            kk = w.shape[-1]
            acc = np.zeros((B, w.shape[0], N, T - kk + 1), f32)
            for j in range(kk):
                acc += np.einsum("oi,bint->bont", w[:, :, 0, j],
                                 x[:, :, :, j:T - kk + 1 + j])
            filts.append((acc + bias[None, :, None, None])[..., -Tp:])
            w, bias = d["gw%d" % k][l], d["gb%d" % k][l]
            acc = np.zeros((B, w.shape[0], N, T - kk + 1), f32)
            for j in range(kk):
                acc += np.einsum("oi,bint->bont", w[:, :, 0, j],
                                 x[:, :, :, j:T - kk + 1 + j])
            gates.append((acc + bias[None, :, None, None])[..., -Tp:])
        filt = np.tanh(np.concatenate(filts, 1))
        gate = 1.0 / (1.0 + np.exp(-np.concatenate(gates, 1)))
        x1 = (filt * gate).astype(f32)
        Q0, Q1, Q2, R0, R1, R2, ub = _fold(d, l)
        p0 = np.einsum("oc,bcnt->bont", Q0 + R0, x1).astype(f32)
        p1 = np.einsum("oc,bcnt->bont", Q1, x1).astype(f32)
        p2 = np.einsum("oc,bcnt->bont", Q2, x1).astype(f32)
        q1 = np.einsum("oc,bcnt->bont", R1, x1).astype(f32)
        q2 = np.einsum("oc,bcnt->bont", R2, x1).astype(f32)
        z = np.einsum("vw,bowt->bovt", adp, p2)
        s1 = 0.5 * (z + p2) + p1
        z1 = np.einsum("vw,bowt->bovt", adp, s1)
        u = p0 + 0.5 * (z1 + s1)
        zz = np.einsum("wv,bowt->bovt", adp, q2)
        s1b = dinv[None, None, :, None] * (zz + q2) + q1
        zz1 = np.einsum("wv,bowt->bovt", adp, s1b)
        u = u + dinv[None, None, :, None] * (zz1 + s1b)
        u = u + ub[None, :, None, None].astype(f32) + x[:, :, :, -Tp:]
        mu = u.mean(axis=(1, 2, 3), keepdims=True)
        var = u.var(axis=(1, 2, 3), keepdims=True)
        x = ((u - mu) / np.sqrt(var + EPS)).astype(f32)
        x = x * d["nw%d" % (l + 1)][None] + d["nb%d" % (l + 1)][None]
    T = x.shape[-1]
    p = np.zeros((TSHORT, T), f32)
    for i in range(TSHORT):
        s = (i * T) // TSHORT
        e = -((-(i + 1) * T) // TSHORT)
        p[i, s:e] = 1.0 / (e - s)
    return np.einsum("st,bcnt->bcsn", p, x).astype(f32)


# ---------------------------------------------------------------- entry
def kernel(**d):
    d = {k: np.asarray(v) for k, v in d.items()}
    adp = _graph_prep(d)
    ln_identity = all(
        bool(np.all(d["nw%d" % j] == 1.0)) and bool(np.all(d["nb%d" % j] == 0.0))
        for j in (1, 2, 3))
    if ln_identity and not _DEV["fail"]:
        out = _device_forward(d, adp)
        if out is not None:
            return out
    return _host_forward(d, adp)


# revision 9
# speedup vs baseline: 4.5013x; 3.5584x over previous
"""Trainium2 Bass kernel for nn_LongTermEncoder (gnn_message_passing).

Sharding: data-parallel over batch B=8 across 8 NeuronCores (adjacency and
all params replicated).  The ENTIRE forward runs on-device in ONE compiled
kernel per core: inception convs (block-diagonal window matmuls), tanh/
sigmoid gating, channel projections (with a ones-row carrying the mixprop
bias), dense adjacency message passing (both directions, 2 hops, folded
through the channel projections exactly as in the reference), residual,
per-sample layernorm, and adaptive average pooling.  The host only builds
the dynamic adjacency (small numpy), folds weights, and reassembles output.

mixprop refactor (exact):  out = Q0 x + A(Q1 x + A(Q2 x)) + B(R1 x + B(R2 x)),
A=(adp+I)/2, B = D^-1(adp^T+I); channel mixing commutes with node mixing.

This container's walrus build rejects instructions with more than ~1 inline
semaphore wait ("Too many sync wait commands"), which Tile emits freely —
every nontrivial Tile kernel fails codegen.  We post-process the BIR JSON
(hoisting excess waits onto EventSemaphore carrier instructions on the same
engine queue, which preserves semantics) via a monkeypatch around
compile_bir_kernel.
"""
import os
import numpy as np

L, GDEP, PA, ALPHA, KTOP, TSHORT, EPS = 3, 2, 0.05, 3.0, 20, 12, 1e-5
KSET = (2, 4, 6, 8)
N, B, RC, CC = 1000, 8, 8, 32
f32 = np.float32

# per-layer time extents
TIN = (168, 161, 154)
TP = (161, 154, 147)
TPMAX = 161

_CACHE_DIR = os.environ.get("JAX_COMPILATION_CACHE_DIR", "/tmp/jaxcache_lte")


# ---------------------------------------------------------------- BIR fix
_LIMITS = {"NoOp": 0, "Drain": 0}
_EV_LIM = 1


def _fix_bir_waits(bir_bytes):
    import orjson
    d = orjson.loads(bir_bytes)
    ctr = 0
    changed = False
    for fn in d.get("functions", []):
        for blk in fn.get("blocks", []):
            newl = []
            for ins in blk.get("instructions", []):
                si = ins.get("sync_info")
                ow = (si or {}).get("on_wait") or []
                lim = _LIMITS.get(ins.get("opcode"), 1)
                if len(ow) > lim:
                    changed = True
                    regw = [w for w in ow if w.get("wait_reg") is not None]
                    immw = [w for w in ow if w.get("wait_reg") is None]
                    keep = (regw + immw)[:lim]
                    excess = (regw + immw)[lim:]
                    ins["sync_info"]["on_wait"] = keep
                    for i in range(0, len(excess), _EV_LIM):
                        ctr += 1
                        newl.append({
                            "debug": ins.get("debug", 0),
                            "engine": ins["engine"],
                            "ins": [], "outs": [],
                            "name": "wsplit-%d" % ctr,
                            "opcode": "EventSemaphore",
                            "sync_info": {"on_update": [],
                                          "on_wait": excess[i:i + _EV_LIM]},
                        })
                newl.append(ins)
            blk["instructions"] = newl
    return orjson.dumps(d) if changed else bir_bytes


_patched = [False]


def _install_birfix():
    if _patched[0]:
        return
    _patched[0] = True
    from concourse import bass2jax, bass_utils
    orig = bass_utils.compile_bir_kernel

    def patched(bir_json, tmpdir, neff_name="file.neff"):
        return orig(_fix_bir_waits(bytes(bir_json)), tmpdir, neff_name)

    bass2jax.compile_bir_kernel = patched


# ---------------------------------------------------------------- host math
def _graph_prep(d):
    emb1, emb2 = d["emb1"], d["emb2"]
    v1 = np.tanh(ALPHA * (emb1 @ d["lin1_w"].T + d["lin1_b"])).astype(f32)
    v2 = np.tanh(ALPHA * (emb2 @ d["lin2_w"].T + d["lin2_b"])).astype(f32)
    a = v1 @ v2.T - v2 @ v1.T
    adj = np.maximum(np.tanh(ALPHA * a), 0.0).astype(f32)
    score = adj + f32(0.01) * d["topk_noise"]
    t1 = np.argpartition(-score, KTOP, axis=1)[:, :KTOP]
    mask = np.zeros((N, N), f32)
    np.put_along_axis(mask, t1, 1.0, axis=1)
    adp = adj * mask
    mv = (1.0 - d["cooldowns"]).astype(f32)
    z = adp * (mv[:, None] * mv[None, :])
    z = z - z.max(axis=1, keepdims=True)
    e = np.exp(z)
    return (e / e.sum(axis=1, keepdims=True)).astype(f32)


def _fold(d, l):
    W = d["g1_w"][l]
    W0, W1, W2 = W[:, :32], W[:, 32:64], W[:, 64:]
    V = d["g2_w"][l]
    V0, V1, V2 = V[:, :32], V[:, 32:64], V[:, 64:]
    al, g = PA, 1.0 - PA
    Q0 = W0 + al * W1 + al * W2
    Q1 = g * W1 + g * al * W2
    Q2 = g * g * W2
    R0 = V0 + al * V1 + al * V2
    R1 = g * V1 + g * al * V2
    R2 = g * g * V2
    ub = d["g1_b"][l] + d["g2_b"][l]
    return Q0, Q1, Q2, R0, R1, R2, ub


# my x1 row order [k8,k6,k4,k2] -> reference channel index (KSET concat order)
_PERM = np.array([24 + i for i in range(8)] + [16 + i for i in range(8)] +
                 [8 + i for i in range(8)] + [0 + i for i in range(8)])


def _prep_weights(d):
    """Per-layer device weights: WA [112,32], WB [48,32], W33 [33,40],
    bias tiles bA/bB [32,1]."""
    import ml_dtypes
    bf16 = ml_dtypes.bfloat16
    out = {}
    for l in range(L):
        WA = np.zeros((112, 48), f32)
        WB = np.zeros((48, 48), f32)
        fw8, fw6 = d["fw8"][l], d["fw6"][l]
        gw8, gw6 = d["gw8"][l], d["gw6"][l]
        fw4, fw2 = d["fw4"][l], d["fw2"][l]
        gw4, gw2 = d["gw4"][l], d["gw2"][l]
        for tap in range(8):
            for ch in range(8):
                r = tap * 8 + ch
                WA[r, 0:8] = fw8[:, ch, 0, tap]
                WA[r, 32:40] = gw8[:, ch, 0, tap]
        for tap in range(2, 8):
            for ch in range(8):
                r = 64 + (tap - 2) * 8 + ch
                WA[r, 8:16] = fw6[:, ch, 0, tap - 2]
                WA[r, 40:48] = gw6[:, ch, 0, tap - 2]
        for tap in range(4, 8):
            for ch in range(8):
                r = (tap - 4) * 8 + ch
                WB[r, 0:8] = fw4[:, ch, 0, tap - 4]
                WB[r, 32:40] = gw4[:, ch, 0, tap - 4]
        for tap in range(6, 8):
            for ch in range(8):
                r = 32 + (tap - 6) * 8 + ch
                WB[r, 8:16] = fw2[:, ch, 0, tap - 6]
                WB[r, 40:48] = gw2[:, ch, 0, tap - 6]
        z8 = np.zeros((16,), f32)
        bA = np.concatenate([d["fb8"][l], d["fb6"][l], z8,
                             d["gb8"][l], d["gb6"][l]]).astype(f32)[:, None]
        bB = np.concatenate([d["fb4"][l], d["fb2"][l], z8,
                             d["gb4"][l], d["gb2"][l]]).astype(f32)[:, None]
        Q0, Q1, Q2, R0, R1, R2, ub = _fold(d, l)
        W33 = np.zeros((65, 40), f32)
        blocks = [Q2, Q1 + 0.5 * Q2, R2, R1, Q0 + R0]
        for bi, Qx in enumerate(blocks):
            qp = Qx[:, _PERM].T          # [32, 8] rows in (k8,k6,k4,k2) order
            W33[0:16, bi * 8:(bi + 1) * 8] = qp[0:16]
            W33[32:48, bi * 8:(bi + 1) * 8] = qp[16:32]
        W33[64, 32:40] = ub
        out["wa%d" % l] = WA.astype(bf16)
        out["wb%d" % l] = WB.astype(bf16)
        out["w33_%d" % l] = W33.astype(bf16)
        out["ba%d" % l] = bA
        out["bb%d" % l] = bB
    return out


def _pool_bounds(t_in):
    bnds = []
    for s in range(TSHORT):
        a = (s * t_in) // TSHORT
        b = -((-(s + 1) * t_in) // TSHORT)
        bnds.append((a, b - a))
    return bnds


# ---------------------------------------------------------------- device
_DEV = {"nc": None, "fail": False}


def _build_nc():
    import concourse.bass as bass
    import concourse.mybir as mybir
    from concourse.tile import TileContext

    bf = mybir.dt.bfloat16
    fp = mybir.dt.float32
    AF = mybir.ActivationFunctionType
    MUL = mybir.AluOpType.mult
    ADD = mybir.AluOpType.add
    SUB = mybir.AluOpType.subtract
    AX = mybir.AxisListType.X

    nc = bass.Bass()
    x0_d = nc.declare_dram_parameter("x0", (N, 8 * TIN[0]), fp, isOutput=False)
    adpT_d = nc.declare_dram_parameter("adpT", (N, N), bf, isOutput=False)
    adp_d = nc.declare_dram_parameter("adp", (N, N), bf, isOutput=False)
    dinv_d = nc.declare_dram_parameter("dinv", (128, 8), fp, isOutput=False)
    wparams = {}
    for l in range(L):
        wparams["wa%d" % l] = nc.declare_dram_parameter("wa%d" % l, (112, 48), bf, isOutput=False)
        wparams["wb%d" % l] = nc.declare_dram_parameter("wb%d" % l, (48, 48), bf, isOutput=False)
        wparams["w33_%d" % l] = nc.declare_dram_parameter("w33_%d" % l, (65, 40), bf, isOutput=False)
        wparams["ba%d" % l] = nc.declare_dram_parameter("ba%d" % l, (48, 1), fp, isOutput=False)
        wparams["bb%d" % l] = nc.declare_dram_parameter("bb%d" % l, (48, 1), fp, isOutput=False)
    out_d = nc.declare_dram_parameter("out", (N, 8 * TSHORT), fp, isOutput=True)

    NT = 8
    rows = [128] * 7 + [104]
    off = [128 * i for i in range(NT)]
    NCH = 64  # conv node-chunk

    with TileContext(nc) as tc:
        with tc.tile_pool(name="glob", bufs=1) as glob, \
             tc.tile_pool(name="dram", bufs=1, space="DRAM") as drp:
            # resident: adjacency, dinv, weights, ones helpers
            aT, aD = [], []
            for k in range(NT):
                t = glob.tile([128, N], bf, tag="aT%d" % k)
                nc.sync.dma_start(out=t[:rows[k], :], in_=adpT_d[off[k]:off[k] + rows[k], :])
                aT.append(t)
                t = glob.tile([128, N], bf, tag="aD%d" % k)
                nc.sync.dma_start(out=t[:rows[k], :], in_=adp_d[off[k]:off[k] + rows[k], :])
                aD.append(t)
            dv = glob.tile([128, 8], fp, tag="dinv")
            nc.sync.dma_start(out=dv[:, :], in_=dinv_d[:, :])
            wt = {}
            for l in range(L):
                for nm, shp in (("wa%d" % l, (112, 48)), ("wb%d" % l, (48, 48)),
                                ("w33_%d" % l, (65, 40))):
                    t = glob.tile([shp[0], shp[1]], bf, tag=nm)
                    nc.sync.dma_start(out=t[:, :], in_=wparams[nm][:, :])
                    wt[nm] = t
                for nm in ("ba%d" % l, "bb%d" % l):
                    t = glob.tile([48, 1], fp, tag=nm)
                    nc.sync.dma_start(out=t[:, :], in_=wparams[nm][:, :])
                    wt[nm] = t
            onescol = glob.tile([128, 1], fp, tag="onescol")
            nc.vector.memset(onescol[:, :], 1.0)
            onesrow = glob.tile([1, 128], fp, tag="onesrow")
            nc.vector.memset(onesrow[:, :], 1.0)
            bc = glob.tile([128, 2], fp, tag="bc")  # (negmu, inv) broadcast

            # DRAM scratch
            pall_s = drp.tile([40, N * TPMAX], bf, tag="pall", name="pall_s")
            s1_s = drp.tile([8, N * TPMAX], bf, tag="s1", name="s1_s")
            s2_s = drp.tile([8, N * TPMAX], bf, tag="s2", name="s2_s")
            xn = [None,
                  drp.tile([N, 8 * TP[0]], fp, tag="xn1", name="xn1"),
                  drp.tile([N, 8 * TP[1]], fp, tag="xn2", name="xn2"),
                  drp.tile([N, 8 * TP[2]], fp, tag="xn3", name="xn3")]

            for l in range(L):
                T, Tp = TIN[l], TP[l]
                xin = x0_d if l == 0 else xn[l]
                F = 8 * Tp

                # ---------------- conv + gating + projection ----------------
                with tc.tile_pool(name="cv%d" % l, bufs=1) as cvp, \
                     tc.tile_pool(name="cvps%d" % l, bufs=1, space="PSUM") as cps:
                    for n0 in range(0, N, NCH):
                        nn = min(NCH, N - n0)
                        cols = nn * Tp
                        xwA = cvp.tile([112, NCH * TPMAX], bf, tag="xwA", name="xwA")
                        xwB = cvp.tile([48, NCH * TPMAX], bf, tag="xwB", name="xwB")
                        xsrc = xin[n0:n0 + nn, :].rearrange("n (c t) -> c n t", t=T)
                        # window loads (cast f32->bf16 via gpsimd)
                        for tap in range(8):
                            nc.gpsimd.dma_start(
                                out=xwA[tap * 8:tap * 8 + 8, :cols].rearrange("r (n t) -> r n t", t=Tp),
                                in_=xsrc[:, :, tap:tap + Tp])
                        for tap in range(2, 8):
                            r = 64 + (tap - 2) * 8
                            nc.gpsimd.dma_start(
                                out=xwA[r:r + 8, :cols].rearrange("r (n t) -> r n t", t=Tp),
                                in_=xsrc[:, :, tap:tap + Tp])
                        for tap in range(4, 8):
                            r = (tap - 4) * 8
                            nc.gpsimd.dma_start(
                                out=xwB[r:r + 8, :cols].rearrange("r (n t) -> r n t", t=Tp),
                                in_=xsrc[:, :, tap:tap + Tp])
                        for tap in range(6, 8):
                            r = 32 + (tap - 6) * 8
                            nc.gpsimd.dma_start(
                                out=xwB[r:r + 8, :cols].rearrange("r (n t) -> r n t", t=Tp),
                                in_=xsrc[:, :, tap:tap + Tp])

                        fsb = cvp.tile([65, NCH * TPMAX], bf, tag="fsb", name="fsb")
                        gsb = cvp.tile([48, NCH * TPMAX], bf, tag="gsb", name="gsb")
                        nc.vector.memset(fsb[:, :cols], 0.0)
                        nc.vector.memset(fsb[64:65, :cols], 1.0)
                        wa, wb = wt["wa%d" % l], wt["wb%d" % l]
                        ba, bb = wt["ba%d" % l], wt["bb%d" % l]
                        for sp0 in range(0, cols, 2048):
                            sw = min(2048, cols - sp0)
                            psA = cps.tile([48, 2048], fp, tag="psA", name="psA")
                            psB = cps.tile([48, 2048], fp, tag="psB", name="psB")
                            for c0 in range(0, sw, 512):
                                cw = min(512, sw - c0)
                                nc.tensor.matmul(psA[:48, c0:c0 + cw], wa[:, :],
                                                 xwA[:, sp0 + c0:sp0 + c0 + cw],
                                                 start=True, stop=True)
                                nc.tensor.matmul(psB[:48, c0:c0 + cw], wb[:, :],
                                                 xwB[:, sp0 + c0:sp0 + c0 + cw],
                                                 start=True, stop=True)
                            sl = slice(sp0, sp0 + sw)
                            nc.scalar.activation(fsb[0:16, sl], psA[0:16, :sw], AF.Tanh, bias=ba[0:16, 0:1])
                            nc.scalar.activation(fsb[32:48, sl], psB[0:16, :sw], AF.Tanh, bias=bb[0:16, 0:1])
                            nc.scalar.activation(gsb[0:16, sl], psA[32:48, :sw], AF.Sigmoid, bias=ba[32:48, 0:1])
                            nc.scalar.activation(gsb[32:48, sl], psB[32:48, :sw], AF.Sigmoid, bias=bb[32:48, 0:1])
                        nc.vector.tensor_mul(fsb[0:16, :cols], fsb[0:16, :cols], gsb[0:16, :cols])
                        nc.vector.tensor_mul(fsb[32:48, :cols], fsb[32:48, :cols], gsb[32:48, :cols])
                        # projection to 40 rows + dump to DRAM
                        w33 = wt["w33_%d" % l]
                        for sp0 in range(0, cols, 2048):
                            sw = min(2048, cols - sp0)
                            psP = cps.tile([40, 2048], fp, tag="psA", name="psP")
                            for c0 in range(0, sw, 512):
                                cw = min(512, sw - c0)
                                nc.tensor.matmul(psP[:40, c0:c0 + cw], w33[:, :],
                                                 fsb[:, sp0 + c0:sp0 + c0 + cw],
                                                 start=True, stop=True)
                            stg = cvp.tile([40, 2048], bf, tag="stg", name="stg")
                            nc.scalar.activation(stg[:, :sw], psP[:, :sw], AF.Copy)
                            nc.sync.dma_start(
                                out=pall_s[:, n0 * Tp + sp0:n0 * Tp + sp0 + sw],
                                in_=stg[:, :sw])

                # ---------------- adjacency passes ----------------
                def pall_blk(b0, k):  # rhs [rows_k, 8*Tp] bf16 view of block b0
                    return pall_s[b0 * 8:b0 * 8 + 8,
                                  off[k] * Tp:(off[k] + rows[k]) * Tp].rearrange(
                                      "c (n t) -> n c t", t=Tp)

                def s_blk(s_s, k):
                    return s_s[:, off[k] * Tp:(off[k] + rows[k]) * Tp].rearrange(
                        "c (n t) -> n c t", t=Tp)

                with tc.tile_pool(name="ad%d" % l, bufs=1) as adp_, \
                     tc.tile_pool(name="adw%d" % l, bufs=3) as adw, \
                     tc.tile_pool(name="adps%d" % l, bufs=1, space="PSUM") as aps:

                    def sweep(lhs, rhs_get, v, ps):
                        for k in range(NT):
                            rt = adw.tile([128, F], bf, tag="rt", name="rt")
                            nc.sync.dma_start(
                                out=rt[:rows[k], :].rearrange("n (c t) -> n c t", t=Tp),
                                in_=rhs_get(k))
                            for c0 in range(0, F, 512):
                                cw = min(512, F - c0)
                                nc.tensor.matmul(ps[:rows[v], c0:c0 + cw],
                                                 lhs[k][:rows[k], off[v]:off[v] + rows[v]],
                                                 rt[:rows[k], c0:c0 + cw],
                                                 start=(k == 0), stop=(k == NT - 1))

                    # pass 1: s1 = 0.5 z + m1 ; s2 = dinv (z' + q2) + q1
                    for v in range(NT):
                        ps = aps.tile([128, F], fp, tag="psZ", name="psZ1")
                        sweep(aT, lambda k: pall_blk(0, k), v, ps)
                        m1v = adw.tile([128, F], fp, tag="in1", name="m1v")
                        nc.gpsimd.dma_start(
                            out=m1v[:rows[v], :].rearrange("n (c t) -> n c t", t=Tp),
                            in_=pall_blk(1, v))
                        s1o = adw.tile([128, F], bf, tag="so", name="s1o")
                        nc.vector.scalar_tensor_tensor(
                            s1o[:rows[v], :], ps[:rows[v], :], 0.5,
                            m1v[:rows[v], :], op0=MUL, op1=ADD)
                        nc.sync.dma_start(out=s_blk(s1_s, v), in_=s1o[:rows[v], :].rearrange("n (c t) -> n c t", t=Tp))

                        ps2 = aps.tile([128, F], fp, tag="psZ", name="psZ2")
                        sweep(aD, lambda k: pall_blk(2, k), v, ps2)
                        q2v = adw.tile([128, F], fp, tag="in1", name="q2v")
                        nc.gpsimd.dma_start(
                            out=q2v[:rows[v], :].rearrange("n (c t) -> n c t", t=Tp),
                            in_=pall_blk(2, v))
                        q1v = adw.tile([128, F], fp, tag="in1", name="q1v")
                        nc.gpsimd.dma_start(
                            out=q1v[:rows[v], :].rearrange("n (c t) -> n c t", t=Tp),
                            in_=pall_blk(3, v))
                        w2 = adw.tile([128, F], fp, tag="w2", name="w2")
                        nc.vector.tensor_add(w2[:rows[v], :], ps2[:rows[v], :], q2v[:rows[v], :])
                        s2o = adw.tile([128, F], bf, tag="so", name="s2o")
                        nc.vector.scalar_tensor_tensor(
                            s2o[:rows[v], :], w2[:rows[v], :], dv[:rows[v], v:v + 1],
                            q1v[:rows[v], :], op0=MUL, op1=ADD)
                        nc.sync.dma_start(out=s_blk(s2_s, v), in_=s2o[:rows[v], :].rearrange("n (c t) -> n c t", t=Tp))

                    # pass 2: u = p0 + 0.5 (z1 + s1) + dinv (z2 + s2) + resid
                    uts = []
                    stats = adp_.tile([128, 16], fp, tag="stats")
                    nc.vector.memset(stats[:, :], 0.0)
                    for v in range(NT):
                        ps = aps.tile([128, F], fp, tag="psZ", name="psZ3")
                        sweep(aT, lambda k: s_blk(s1_s, k), v, ps)
                        s1v = adw.tile([128, F], fp, tag="in1", name="s1v")
                        nc.gpsimd.dma_start(out=s1v[:rows[v], :].rearrange("n (c t) -> n c t", t=Tp), in_=s_blk(s1_s, v))
                        p0v = adw.tile([128, F], fp, tag="in1", name="p0v")
                        nc.gpsimd.dma_start(out=p0v[:rows[v], :].rearrange("n (c t) -> n c t", t=Tp), in_=pall_blk(4, v))
                        w1 = adw.tile([128, F], fp, tag="w2", name="w1b")
                        nc.vector.tensor_add(w1[:rows[v], :], ps[:rows[v], :], s1v[:rows[v], :])
                        ut = adp_.tile([128, F], fp, tag="ut%d" % v, name="ut%d" % v)
                        nc.vector.scalar_tensor_tensor(
                            ut[:rows[v], :], w1[:rows[v], :], 0.5,
                            p0v[:rows[v], :], op0=MUL, op1=ADD)

                        ps2 = aps.tile([128, F], fp, tag="psZ", name="psZ4")
                        sweep(aD, lambda k: s_blk(s2_s, k), v, ps2)
                        s2v = adw.tile([128, F], fp, tag="in1", name="s2v")
                        nc.gpsimd.dma_start(out=s2v[:rows[v], :].rearrange("n (c t) -> n c t", t=Tp), in_=s_blk(s2_s, v))
                        w2 = adw.tile([128, F], fp, tag="w2", name="w2b")
                        nc.vector.tensor_add(w2[:rows[v], :], ps2[:rows[v], :], s2v[:rows[v], :])
                        nc.vector.scalar_tensor_tensor(
                            ut[:rows[v], :], w2[:rows[v], :], dv[:rows[v], v:v + 1],
                            ut[:rows[v], :], op0=MUL, op1=ADD)
                        # residual (last Tp of each channel of x_in)
                        xr = adw.tile([128, F], fp, tag="xr", name="xr")
                        nc.sync.dma_start(
                            out=xr[:rows[v], :].rearrange("n (c t) -> n c t", t=Tp),
                            in_=xin[off[v]:off[v] + rows[v], :].rearrange(
                                "n (c t) -> n c t", t=T)[:, :, T - Tp:])
                        nc.vector.tensor_add(ut[:rows[v], :], ut[:rows[v], :], xr[:rows[v], :])
                        # stats partials
                        nc.vector.reduce_sum(stats[:rows[v], 2 * v:2 * v + 1], ut[:rows[v], :], axis=AX)
                        usq = adw.tile([128, F], fp, tag="w2", name="usq")
                        nc.vector.tensor_mul(usq[:rows[v], :], ut[:rows[v], :], ut[:rows[v], :])
                        nc.vector.reduce_sum(stats[:rows[v], 2 * v + 1:2 * v + 2], usq[:rows[v], :], axis=AX)
                        uts.append(ut)

                    # layernorm stats across partitions
                    psS = aps.tile([1, 16], fp, tag="psS", name="psS")
                    nc.tensor.matmul(psS[:1, :16], onescol[:, :], stats[:, :], start=True, stop=True)
                    CNT = float(8 * N * Tp)
                    sm = adp_.tile([1, 8], fp, tag="sm")
                    nc.vector.reduce_sum(sm[:1, 0:1], psS[0:1, :].rearrange("p (v two) -> p two v", two=2)[:, 0:1, :], axis=AX)
                    nc.vector.reduce_sum(sm[:1, 1:2], psS[0:1, :].rearrange("p (v two) -> p two v", two=2)[:, 1:2, :], axis=AX)
                    # mu = sm0/CNT ; var = sm1/CNT - mu^2 ; inv = 1/sqrt(var+EPS)
                    nc.vector.tensor_scalar_mul(sm[:1, 2:3], sm[:1, 0:1], 1.0 / CNT)   # mu
                    nc.vector.tensor_scalar_mul(sm[:1, 3:4], sm[:1, 1:2], 1.0 / CNT)   # E[x^2]
                    nc.vector.tensor_mul(sm[:1, 4:5], sm[:1, 2:3], sm[:1, 2:3])        # mu^2
                    nc.vector.tensor_sub(sm[:1, 5:6], sm[:1, 3:4], sm[:1, 4:5])        # var
                    nc.vector.tensor_scalar_add(sm[:1, 5:6], sm[:1, 5:6], float(EPS))
                    nc.scalar.activation(sm[:1, 6:7], sm[:1, 5:6], AF.Sqrt)
                    nc.vector.reciprocal(sm[:1, 7:8], sm[:1, 6:7])                     # inv
                    nc.vector.tensor_scalar_mul(sm[:1, 2:3], sm[:1, 2:3], -1.0)        # -mu
                    bc2 = adp_.tile([1, 2], fp, tag="bc2")
                    nc.vector.tensor_copy(bc2[:1, 0:1], sm[:1, 2:3])
                    nc.vector.tensor_copy(bc2[:1, 1:2], sm[:1, 7:8])
                    psb = aps.tile([128, 2], fp, tag="psS", name="psb")
                    nc.tensor.matmul(psb[:, :2], onesrow[:, :], bc2[:1, :2], start=True, stop=True)
                    nc.scalar.activation(bc[:, :2], psb[:, :2], AF.Copy)

                    # apply LN, write x_{l+1} (or pooling input) to DRAM
                    for v in range(NT):
                        xo = adw.tile([128, F], fp, tag="xo", name="xo")
                        nc.vector.tensor_scalar(
                            xo[:rows[v], :], uts[v][:rows[v], :],
                            bc[:rows[v], 0:1], bc[:rows[v], 1:2], op0=ADD, op1=MUL)
                        if l < L - 1:
                            nc.sync.dma_start(out=xn[l + 1][off[v]:off[v] + rows[v], :], in_=xo[:rows[v], :])
                        else:
                            nc.sync.dma_start(out=xn[3][off[v]:off[v] + rows[v], :], in_=xo[:rows[v], :])

            # ---------------- adaptive avg pooling ----------------
            with tc.tile_pool(name="pool", bufs=2) as plp:
                bnds = _pool_bounds(TP[2])
                for v in range(NT):
                    xt = plp.tile([128, 8 * TP[2]], fp, tag="xt", name="xt")
                    nc.sync.dma_start(out=xt[:rows[v], :], in_=xn[3][off[v]:off[v] + rows[v], :])
                    ot = plp.tile([128, 8 * TSHORT], fp, tag="ot", name="ot")
                    xv = xt[:rows[v], :].rearrange("n (c t) -> n c t", t=TP[2])
                    for s, (a, w) in enumerate(bnds):
                        nc.vector.reduce_sum(ot[:rows[v], s * 8:(s + 1) * 8],
                                             xv[:, :, a:a + w], axis=AX)
                    nc.sync.dma_start(out=out_d[off[v]:off[v] + rows[v], :], in_=ot[:rows[v], :8 * TSHORT])
    return nc


def _device_forward(d, adp):
    try:
        import jax
        try:
            jax.config.update("jax_compilation_cache_dir", _CACHE_DIR)
            jax.config.update("jax_persistent_cache_min_compile_time_secs", 0.0)
            jax.config.update("jax_persistent_cache_min_entry_size_bytes", -1)
        except Exception:
            pass
        _install_birfix()
        from concourse.bass_utils import run_bass_kernel_spmd
        import ml_dtypes
        bf16 = ml_dtypes.bfloat16

        dinv = (1.0 / (1.0 + adp.sum(axis=0))).astype(f32)
        dpad = np.zeros((1024,), f32)
        dpad[:N] = dinv
        dmat = np.ascontiguousarray(dpad.reshape(8, 128).T)

        # start conv on host: x0 [B, N, 8, T]
        inp = d["input"]
        xf = inp.transpose(0, 2, 3, 1).reshape(-1, 2) @ d["start_w"].T
        xf += d["start_b"][None, :]
        x0 = np.ascontiguousarray(
            xf.reshape(B, N, TIN[0], 8).transpose(0, 1, 3, 2)).reshape(B, N, 8 * TIN[0])

        wts = _prep_weights(d)
        adpT_b = np.ascontiguousarray(adp.T).astype(bf16)
        adp_b = adp.astype(bf16)
        if _DEV["nc"] is None:
            _DEV["nc"] = _build_nc()
        in_maps = []
        for b in range(B):
            m = {"x0": x0[b].astype(f32), "adpT": adpT_b, "adp": adp_b,
                 "dinv": dmat}
            m.update(wts)
            in_maps.append(m)
        res = run_bass_kernel_spmd(_DEV["nc"], in_maps, list(range(B)))
        scale = np.array([1.0 / w for (_, w) in _pool_bounds(TP[2])], f32)
        outs = []
        for b in range(B):
            ob = np.asarray(res.results[b]["out"], f32).reshape(N, TSHORT, 8)
            outs.append(ob.transpose(2, 1, 0) * scale[None, :, None])
        return np.stack(outs, 0)
    except Exception:
        import traceback
        traceback.print_exc()
        _DEV["fail"] = True
        return None


# ---------------------------------------------------------------- host fallback
def _host_forward(d, adp):
    dinv = (1.0 / (1.0 + adp.sum(axis=0))).astype(f32)
    x = np.einsum("bint,oi->bont", d["input"], d["start_w"]).astype(f32) + \
        d["start_b"][None, :, None, None]
    for l in range(L):
        T = x.shape[-1]
        Tp = T - 7
        filts, gates = [], []
        for k in KSET:
            w, bias = d["fw%d" % k][l], d["fb%d" % k][l]
            kk = w.shape[-1]
            acc = np.zeros((B, w.shape[0], N, T - kk + 1), f32)
            for j in range(kk):
                acc += np.einsum("oi,bint->bont", w[:, :, 0, j],
                                 x[:, :, :, j:T - kk + 1 + j])
            filts.append((acc + bias[None, :, None, None])[..., -Tp:])
            w, bias = d["gw%d" % k][l], d["gb%d" % k][l]
            acc = np.zeros((B, w.shape[0], N, T - kk + 1), f32)
            for j in range(kk):
                acc += np.einsum("oi,bint->bont", w[:, :, 0, j],
                                 x[:, :, :, j:T - kk + 1 + j])
            gates.append((acc + bias[None, :, None, None])[..., -Tp:])
        filt = np.tanh(np.concatenate(filts, 1))
        gate = 1.0 / (1.0 + np.exp(-np.concatenate(gates, 1)))
        x1 = (filt * gate).astype(f32)
        Q0, Q1, Q2, R0, R1, R2, ub = _fold(d, l)
        p0 = np.einsum("oc,bcnt->bont", Q0 + R0, x1).astype(f32)
        p1 = np.einsum("oc,bcnt->bont", Q1, x1).astype(f32)
        p2 = np.einsum("oc,bcnt->bont", Q2, x1).astype(f32)
        q1 = np.einsum("oc,bcnt->bont", R1, x1).astype(f32)
        q2 = np.einsum("oc,bcnt->bont", R2, x1).astype(f32)
        z = np.einsum("vw,bowt->bovt", adp, p2)
        s1 = 0.5 * (z + p2) + p1
        z1 = np.einsum("vw,bowt->bovt", adp, s1)
        u = p0 + 0.5 * (z1 + s1)
        zz = np.einsum("wv,bowt->bovt", adp, q2)
        s1b = dinv[None, None, :, None] * (zz + q2) + q1
        zz1 = np.einsum("wv,bowt->bovt", adp, s1b)
        u = u + dinv[None, None, :, None] * (zz1 + s1b)
        u = u + ub[None, :, None, None].astype(f32) + x[:, :, :, -Tp:]
        mu = u.mean(axis=(1, 2, 3), keepdims=True)
        var = u.var(axis=(1, 2, 3), keepdims=True)
        x = ((u - mu) / np.sqrt(var + EPS)).astype(f32)
        x = x * d["nw%d" % (l + 1)][None] + d["nb%d" % (l + 1)][None]
    T = x.shape[-1]
    p = np.zeros((TSHORT, T), f32)
    for i in range(TSHORT):
        s = (i * T) // TSHORT
        e = -((-(i + 1) * T) // TSHORT)
        p[i, s:e] = 1.0 / (e - s)
    return np.einsum("st,bcnt->bcsn", p, x).astype(f32)


# ---------------------------------------------------------------- entry
_CHILD_KEYS = ("input", "start_w", "start_b",
               "fw2", "fw4", "fw6", "fw8", "fb2", "fb4", "fb6", "fb8",
               "gw2", "gw4", "gw6", "gw8", "gb2", "gb4", "gb6", "gb8",
               "g1_w", "g1_b", "g2_w", "g2_b")


def _child_main(inp, outp):
    z = np.load(inp)
    d = {k: z[k] for k in z.files if k != "adp"}
    out = _device_forward(d, z["adp"])
    if out is None:
        raise SystemExit(3)
    np.savez(outp, out=out)


def _device_subprocess(d, adp):
    """Run the device path in a PYTHONHASHSEED=0 child so the jax
    persistent-cache key is deterministic (hash randomization perturbs it)."""
    import subprocess, sys, tempfile, os as _os
    try:
        tmp = tempfile.mkdtemp()
        inp = _os.path.join(tmp, "in.npz")
        outp = _os.path.join(tmp, "out.npz")
        np.savez(inp, adp=adp, **{k: d[k] for k in _CHILD_KEYS})
        here = _os.path.dirname(_os.path.abspath(__file__))
        code = ("import sys; sys.path.insert(0, %r); "
                "import kernel; kernel._child_main(%r, %r)" % (here, inp, outp))
        env = dict(_os.environ)
        env["PYTHONHASHSEED"] = "0"
        r = subprocess.run([sys.executable, "-c", code], env=env, timeout=560,
                           stdout=subprocess.DEVNULL, stderr=subprocess.DEVNULL)
        if r.returncode != 0 or not _os.path.exists(outp):
            return None
        return np.load(outp)["out"]
    except Exception:
        return None


def kernel(**d):
    import sys
    d = {k: np.asarray(v) for k, v in d.items()}
    adp = _graph_prep(d)
    ln_identity = all(
        bool(np.all(d["nw%d" % j] == 1.0)) and bool(np.all(d["nb%d" % j] == 0.0))
        for j in (1, 2, 3))
    if ln_identity and not _DEV["fail"]:
        out = None
        if sys.flags.hash_randomization == 0 or "jax" in sys.modules:
            out = _device_forward(d, adp)
        else:
            out = _device_subprocess(d, adp)
            if out is None:
                out = _device_forward(d, adp)
        if out is not None:
            return out
    return _host_forward(d, adp)


# revision 10
# speedup vs baseline: 4.9086x; 1.0905x over previous
"""Trainium2 Bass kernel for nn_LongTermEncoder (gnn_message_passing).

Sharding: data-parallel over batch B=8 across 8 NeuronCores (adjacency and
all params replicated).  The ENTIRE forward runs on-device in ONE compiled
kernel per core: inception convs (block-diagonal window matmuls), tanh/
sigmoid gating, channel projections (with a ones-row carrying the mixprop
bias), dense adjacency message passing (both directions, 2 hops, folded
through the channel projections exactly as in the reference), residual,
per-sample layernorm, and adaptive average pooling.  The host only builds
the dynamic adjacency (small numpy), folds weights, and reassembles output.

mixprop refactor (exact):  out = Q0 x + A(Q1 x + A(Q2 x)) + B(R1 x + B(R2 x)),
A=(adp+I)/2, B = D^-1(adp^T+I); channel mixing commutes with node mixing.

This container's walrus build rejects instructions with more than ~1 inline
semaphore wait ("Too many sync wait commands"), which Tile emits freely —
every nontrivial Tile kernel fails codegen.  We post-process the BIR JSON
(hoisting excess waits onto EventSemaphore carrier instructions on the same
engine queue, which preserves semantics) via a monkeypatch around
compile_bir_kernel.
"""
import os
import numpy as np

L, GDEP, PA, ALPHA, KTOP, TSHORT, EPS = 3, 2, 0.05, 3.0, 20, 12, 1e-5
KSET = (2, 4, 6, 8)
N, B, RC, CC = 1000, 8, 8, 32
f32 = np.float32

# per-layer time extents
TIN = (168, 161, 154)
TP = (161, 154, 147)
TPMAX = 161

_CACHE_DIR = os.environ.get("JAX_COMPILATION_CACHE_DIR", "/tmp/jaxcache_lte")


# ---------------------------------------------------------------- BIR fix
_LIMITS = {"NoOp": 0, "Drain": 0}
_EV_LIM = 1


def _fix_bir_waits(bir_bytes):
    import orjson
    d = orjson.loads(bir_bytes)
    ctr = 0
    changed = False
    for fn in d.get("functions", []):
        for blk in fn.get("blocks", []):
            newl = []
            for ins in blk.get("instructions", []):
                si = ins.get("sync_info")
                ow = (si or {}).get("on_wait") or []
                lim = _LIMITS.get(ins.get("opcode"), 1)
                if len(ow) > lim:
                    changed = True
                    regw = [w for w in ow if w.get("wait_reg") is not None]
                    immw = [w for w in ow if w.get("wait_reg") is None]
                    keep = (regw + immw)[:lim]
                    excess = (regw + immw)[lim:]
                    ins["sync_info"]["on_wait"] = keep
                    for i in range(0, len(excess), _EV_LIM):
                        ctr += 1
                        newl.append({
                            "debug": ins.get("debug", 0),
                            "engine": ins["engine"],
                            "ins": [], "outs": [],
                            "name": "wsplit-%d" % ctr,
                            "opcode": "EventSemaphore",
                            "sync_info": {"on_update": [],
                                          "on_wait": excess[i:i + _EV_LIM]},
                        })
                newl.append(ins)
            blk["instructions"] = newl
    return orjson.dumps(d) if changed else bir_bytes


_patched = [False]


def _install_birfix():
    if _patched[0]:
        return
    _patched[0] = True
    from concourse import bass2jax, bass_utils
    orig = bass_utils.compile_bir_kernel

    def patched(bir_json, tmpdir, neff_name="file.neff"):
        return orig(_fix_bir_waits(bytes(bir_json)), tmpdir, neff_name)

    bass2jax.compile_bir_kernel = patched


# ---------------------------------------------------------------- host math
def _graph_prep(d):
    emb1, emb2 = d["emb1"], d["emb2"]
    v1 = np.tanh(ALPHA * (emb1 @ d["lin1_w"].T + d["lin1_b"])).astype(f32)
    v2 = np.tanh(ALPHA * (emb2 @ d["lin2_w"].T + d["lin2_b"])).astype(f32)
    a = v1 @ v2.T - v2 @ v1.T
    adj = np.maximum(np.tanh(ALPHA * a), 0.0).astype(f32)
    score = adj + f32(0.01) * d["topk_noise"]
    t1 = np.argpartition(-score, KTOP, axis=1)[:, :KTOP]
    mask = np.zeros((N, N), f32)
    np.put_along_axis(mask, t1, 1.0, axis=1)
    adp = adj * mask
    mv = (1.0 - d["cooldowns"]).astype(f32)
    z = adp * (mv[:, None] * mv[None, :])
    z = z - z.max(axis=1, keepdims=True)
    e = np.exp(z)
    return (e / e.sum(axis=1, keepdims=True)).astype(f32)


def _fold(d, l):
    W = d["g1_w"][l]
    W0, W1, W2 = W[:, :32], W[:, 32:64], W[:, 64:]
    V = d["g2_w"][l]
    V0, V1, V2 = V[:, :32], V[:, 32:64], V[:, 64:]
    al, g = PA, 1.0 - PA
    Q0 = W0 + al * W1 + al * W2
    Q1 = g * W1 + g * al * W2
    Q2 = g * g * W2
    R0 = V0 + al * V1 + al * V2
    R1 = g * V1 + g * al * V2
    R2 = g * g * V2
    ub = d["g1_b"][l] + d["g2_b"][l]
    return Q0, Q1, Q2, R0, R1, R2, ub


# my x1 row order [k8,k6,k4,k2] -> reference channel index (KSET concat order)
_PERM = np.array([24 + i for i in range(8)] + [16 + i for i in range(8)] +
                 [8 + i for i in range(8)] + [0 + i for i in range(8)])


def _prep_weights(d):
    """Per-layer device weights: WA [112,32], WB [48,32], W33 [33,40],
    bias tiles bA/bB [32,1]."""
    import ml_dtypes
    bf16 = ml_dtypes.bfloat16
    out = {}
    for l in range(L):
        WA = np.zeros((112, 48), f32)
        WB = np.zeros((48, 48), f32)
        fw8, fw6 = d["fw8"][l], d["fw6"][l]
        gw8, gw6 = d["gw8"][l], d["gw6"][l]
        fw4, fw2 = d["fw4"][l], d["fw2"][l]
        gw4, gw2 = d["gw4"][l], d["gw2"][l]
        for tap in range(8):
            for ch in range(8):
                r = tap * 8 + ch
                WA[r, 0:8] = fw8[:, ch, 0, tap]
                WA[r, 32:40] = gw8[:, ch, 0, tap]
        for tap in range(2, 8):
            for ch in range(8):
                r = 64 + (tap - 2) * 8 + ch
                WA[r, 8:16] = fw6[:, ch, 0, tap - 2]
                WA[r, 40:48] = gw6[:, ch, 0, tap - 2]
        for tap in range(4, 8):
            for ch in range(8):
                r = (tap - 4) * 8 + ch
                WB[r, 0:8] = fw4[:, ch, 0, tap - 4]
                WB[r, 32:40] = gw4[:, ch, 0, tap - 4]
        for tap in range(6, 8):
            for ch in range(8):
                r = 32 + (tap - 6) * 8 + ch
                WB[r, 8:16] = fw2[:, ch, 0, tap - 6]
                WB[r, 40:48] = gw2[:, ch, 0, tap - 6]
        z8 = np.zeros((16,), f32)
        bA = np.concatenate([d["fb8"][l], d["fb6"][l], z8,
                             d["gb8"][l], d["gb6"][l]]).astype(f32)[:, None]
        bB = np.concatenate([d["fb4"][l], d["fb2"][l], z8,
                             d["gb4"][l], d["gb2"][l]]).astype(f32)[:, None]
        Q0, Q1, Q2, R0, R1, R2, ub = _fold(d, l)
        W33 = np.zeros((65, 40), f32)
        blocks = [Q2, Q1 + 0.5 * Q2, R2, R1, Q0 + R0]
        for bi, Qx in enumerate(blocks):
            qp = Qx[:, _PERM].T          # [32, 8] rows in (k8,k6,k4,k2) order
            W33[0:16, bi * 8:(bi + 1) * 8] = qp[0:16]
            W33[32:48, bi * 8:(bi + 1) * 8] = qp[16:32]
        W33[64, 32:40] = ub
        out["wa%d" % l] = WA.astype(bf16)
        out["wb%d" % l] = WB.astype(bf16)
        out["w33_%d" % l] = W33.astype(bf16)
        out["ba%d" % l] = bA
        out["bb%d" % l] = bB
    return out


def _pool_bounds(t_in):
    bnds = []
    for s in range(TSHORT):
        a = (s * t_in) // TSHORT
        b = -((-(s + 1) * t_in) // TSHORT)
        bnds.append((a, b - a))
    return bnds


# ---------------------------------------------------------------- device
_DEV = {"nc": None, "fail": False}


def _build_nc():
    import concourse.bass as bass
    import concourse.mybir as mybir
    from concourse.tile import TileContext

    bf = mybir.dt.bfloat16
    fp = mybir.dt.float32
    AF = mybir.ActivationFunctionType
    MUL = mybir.AluOpType.mult
    ADD = mybir.AluOpType.add
    SUB = mybir.AluOpType.subtract
    AX = mybir.AxisListType.X

    nc = bass.Bass()
    x0_d = nc.declare_dram_parameter("x0", (N, 8 * TIN[0]), fp, isOutput=False)
    adpT_d = nc.declare_dram_parameter("adpT", (N, N), bf, isOutput=False)
    adp_d = nc.declare_dram_parameter("adp", (N, N), bf, isOutput=False)
    dinv_d = nc.declare_dram_parameter("dinv", (128, 8), fp, isOutput=False)
    wparams = {}
    for l in range(L):
        wparams["wa%d" % l] = nc.declare_dram_parameter("wa%d" % l, (112, 48), bf, isOutput=False)
        wparams["wb%d" % l] = nc.declare_dram_parameter("wb%d" % l, (48, 48), bf, isOutput=False)
        wparams["w33_%d" % l] = nc.declare_dram_parameter("w33_%d" % l, (65, 40), bf, isOutput=False)
        wparams["ba%d" % l] = nc.declare_dram_parameter("ba%d" % l, (48, 1), fp, isOutput=False)
        wparams["bb%d" % l] = nc.declare_dram_parameter("bb%d" % l, (48, 1), fp, isOutput=False)
    out_d = nc.declare_dram_parameter("out", (N, 8 * TSHORT), fp, isOutput=True)

    NT = 8
    rows = [128] * 7 + [104]
    off = [128 * i for i in range(NT)]
    NCH = 64  # conv node-chunk

    with TileContext(nc) as tc:
        with tc.tile_pool(name="glob", bufs=1) as glob, \
             tc.tile_pool(name="dram", bufs=1, space="DRAM") as drp:
            # resident: adjacency, dinv, weights, ones helpers
            aT, aD = [], []
            for k in range(NT):
                t = glob.tile([128, N], bf, tag="aT%d" % k)
                nc.sync.dma_start(out=t[:rows[k], :], in_=adpT_d[off[k]:off[k] + rows[k], :])
                aT.append(t)
                t = glob.tile([128, N], bf, tag="aD%d" % k)
                nc.sync.dma_start(out=t[:rows[k], :], in_=adp_d[off[k]:off[k] + rows[k], :])
                aD.append(t)
            dv = glob.tile([128, 8], fp, tag="dinv")
            nc.sync.dma_start(out=dv[:, :], in_=dinv_d[:, :])
            wt = {}
            for l in range(L):
                for nm, shp in (("wa%d" % l, (112, 48)), ("wb%d" % l, (48, 48)),
                                ("w33_%d" % l, (65, 40))):
                    t = glob.tile([shp[0], shp[1]], bf, tag=nm)
                    nc.sync.dma_start(out=t[:, :], in_=wparams[nm][:, :])
                    wt[nm] = t
                for nm in ("ba%d" % l, "bb%d" % l):
                    t = glob.tile([48, 1], fp, tag=nm)
                    nc.sync.dma_start(out=t[:, :], in_=wparams[nm][:, :])
                    wt[nm] = t
            onescol = glob.tile([128, 1], fp, tag="onescol")
            nc.vector.memset(onescol[:, :], 1.0)
            onesrow = glob.tile([1, 128], fp, tag="onesrow")
            nc.vector.memset(onesrow[:, :], 1.0)
            bc = glob.tile([128, 2], fp, tag="bc")  # (negmu, inv) broadcast

            # DRAM scratch
            pall_s = drp.tile([40, N * TPMAX], bf, tag="pall", name="pall_s")
            s1_s = drp.tile([8, N * TPMAX], bf, tag="s1", name="s1_s")
            s2_s = drp.tile([8, N * TPMAX], bf, tag="s2", name="s2_s")
            xn = [None,
                  drp.tile([N, 8 * TP[0]], fp, tag="xn1", name="xn1"),
                  drp.tile([N, 8 * TP[1]], fp, tag="xn2", name="xn2"),
                  drp.tile([N, 8 * TP[2]], fp, tag="xn3", name="xn3")]

            for l in range(L):
                T, Tp = TIN[l], TP[l]
                xin = x0_d if l == 0 else xn[l]
                F = 8 * Tp

                # ---------------- conv + gating + projection ----------------
                with tc.tile_pool(name="cv%d" % l, bufs=1) as cvp, \
                     tc.tile_pool(name="cvps%d" % l, bufs=1, space="PSUM") as cps:
                    for n0 in range(0, N, NCH):
                        nn = min(NCH, N - n0)
                        cols = nn * Tp
                        xwA = cvp.tile([112, NCH * TPMAX], bf, tag="xwA", name="xwA")
                        xwB = cvp.tile([48, NCH * TPMAX], bf, tag="xwB", name="xwB")
                        xsrc = xin[n0:n0 + nn, :].rearrange("n (c t) -> c n t", t=T)
                        # window loads (cast f32->bf16 via gpsimd)
                        for tap in range(8):
                            nc.gpsimd.dma_start(
                                out=xwA[tap * 8:tap * 8 + 8, :cols].rearrange("r (n t) -> r n t", t=Tp),
                                in_=xsrc[:, :, tap:tap + Tp])
                        for tap in range(2, 8):
                            r = 64 + (tap - 2) * 8
                            nc.gpsimd.dma_start(
                                out=xwA[r:r + 8, :cols].rearrange("r (n t) -> r n t", t=Tp),
                                in_=xsrc[:, :, tap:tap + Tp])
                        for tap in range(4, 8):
                            r = (tap - 4) * 8
                            nc.gpsimd.dma_start(
                                out=xwB[r:r + 8, :cols].rearrange("r (n t) -> r n t", t=Tp),
                                in_=xsrc[:, :, tap:tap + Tp])
                        for tap in range(6, 8):
                            r = 32 + (tap - 6) * 8
                            nc.gpsimd.dma_start(
                                out=xwB[r:r + 8, :cols].rearrange("r (n t) -> r n t", t=Tp),
                                in_=xsrc[:, :, tap:tap + Tp])

                        fsb = cvp.tile([65, NCH * TPMAX], bf, tag="fsb", name="fsb")
                        gsb = cvp.tile([48, NCH * TPMAX], bf, tag="gsb", name="gsb")
                        nc.vector.memset(fsb[:, :cols], 0.0)
                        nc.vector.memset(fsb[64:65, :cols], 1.0)
                        wa, wb = wt["wa%d" % l], wt["wb%d" % l]
                        ba, bb = wt["ba%d" % l], wt["bb%d" % l]
                        for sp0 in range(0, cols, 2048):
                            sw = min(2048, cols - sp0)
                            psA = cps.tile([48, 2048], fp, tag="psA", name="psA")
                            psB = cps.tile([48, 2048], fp, tag="psB", name="psB")
                            for c0 in range(0, sw, 512):
                                cw = min(512, sw - c0)
                                nc.tensor.matmul(psA[:48, c0:c0 + cw], wa[:, :],
                                                 xwA[:, sp0 + c0:sp0 + c0 + cw],
                                                 start=True, stop=True)
                                nc.tensor.matmul(psB[:48, c0:c0 + cw], wb[:, :],
                                                 xwB[:, sp0 + c0:sp0 + c0 + cw],
                                                 start=True, stop=True)
                            sl = slice(sp0, sp0 + sw)
                            nc.scalar.activation(fsb[0:16, sl], psA[0:16, :sw], AF.Tanh, bias=ba[0:16, 0:1])
                            nc.scalar.activation(fsb[32:48, sl], psB[0:16, :sw], AF.Tanh, bias=bb[0:16, 0:1])
                            nc.scalar.activation(gsb[0:16, sl], psA[32:48, :sw], AF.Sigmoid, bias=ba[32:48, 0:1])
                            nc.scalar.activation(gsb[32:48, sl], psB[32:48, :sw], AF.Sigmoid, bias=bb[32:48, 0:1])
                        nc.vector.tensor_mul(fsb[0:16, :cols], fsb[0:16, :cols], gsb[0:16, :cols])
                        nc.vector.tensor_mul(fsb[32:48, :cols], fsb[32:48, :cols], gsb[32:48, :cols])
                        # projection to 40 rows + dump to DRAM
                        w33 = wt["w33_%d" % l]
                        for sp0 in range(0, cols, 2048):
                            sw = min(2048, cols - sp0)
                            psP = cps.tile([40, 2048], fp, tag="psA", name="psP")
                            for c0 in range(0, sw, 512):
                                cw = min(512, sw - c0)
                                nc.tensor.matmul(psP[:40, c0:c0 + cw], w33[:, :],
                                                 fsb[:, sp0 + c0:sp0 + c0 + cw],
                                                 start=True, stop=True)
                            stg = cvp.tile([40, 2048], bf, tag="stg", name="stg")
                            nc.scalar.activation(stg[:, :sw], psP[:, :sw], AF.Copy)
                            nc.sync.dma_start(
                                out=pall_s[:, n0 * Tp + sp0:n0 * Tp + sp0 + sw],
                                in_=stg[:, :sw])

                # ---------------- adjacency passes ----------------
                def pall_blk(b0, k):  # rhs [rows_k, 8*Tp] bf16 view of block b0
                    return pall_s[b0 * 8:b0 * 8 + 8,
                                  off[k] * Tp:(off[k] + rows[k]) * Tp].rearrange(
                                      "c (n t) -> n c t", t=Tp)

                def s_blk(s_s, k):
                    return s_s[:, off[k] * Tp:(off[k] + rows[k]) * Tp].rearrange(
                        "c (n t) -> n c t", t=Tp)

                with tc.tile_pool(name="ad%d" % l, bufs=1) as adp_, \
                     tc.tile_pool(name="adw%d" % l, bufs=3) as adw, \
                     tc.tile_pool(name="adps%d" % l, bufs=1, space="PSUM") as aps:

                    def sweep(lhs, rhs_get, v, ps):
                        for k in range(NT):
                            rt = adw.tile([128, F], bf, tag="rt", name="rt")
                            nc.sync.dma_start(
                                out=rt[:rows[k], :].rearrange("n (c t) -> n c t", t=Tp),
                                in_=rhs_get(k))
                            for c0 in range(0, F, 512):
                                cw = min(512, F - c0)
                                nc.tensor.matmul(ps[:rows[v], c0:c0 + cw],
                                                 lhs[k][:rows[k], off[v]:off[v] + rows[v]],
                                                 rt[:rows[k], c0:c0 + cw],
                                                 start=(k == 0), stop=(k == NT - 1))

                    # pass 1: s1 = 0.5 z + m1 ; s2 = dinv (z' + q2) + q1
                    for v in range(NT):
                        ps = aps.tile([128, F], fp, tag="psZ", name="psZ1")
                        sweep(aT, lambda k: pall_blk(0, k), v, ps)
                        m1v = adw.tile([128, F], fp, tag="in1", name="m1v")
                        nc.gpsimd.dma_start(
                            out=m1v[:rows[v], :].rearrange("n (c t) -> n c t", t=Tp),
                            in_=pall_blk(1, v))
                        s1o = adw.tile([128, F], bf, tag="so", name="s1o")
                        nc.vector.scalar_tensor_tensor(
                            s1o[:rows[v], :], ps[:rows[v], :], 0.5,
                            m1v[:rows[v], :], op0=MUL, op1=ADD)
                        nc.sync.dma_start(out=s_blk(s1_s, v), in_=s1o[:rows[v], :].rearrange("n (c t) -> n c t", t=Tp))

                        ps2 = aps.tile([128, F], fp, tag="psZ", name="psZ2")
                        sweep(aD, lambda k: pall_blk(2, k), v, ps2)
                        q2v = adw.tile([128, F], fp, tag="in1", name="q2v")
                        nc.gpsimd.dma_start(
                            out=q2v[:rows[v], :].rearrange("n (c t) -> n c t", t=Tp),
                            in_=pall_blk(2, v))
                        q1v = adw.tile([128, F], fp, tag="in1", name="q1v")
                        nc.gpsimd.dma_start(
                            out=q1v[:rows[v], :].rearrange("n (c t) -> n c t", t=Tp),
                            in_=pall_blk(3, v))
                        w2 = adw.tile([128, F], fp, tag="w2", name="w2")
                        nc.vector.tensor_add(w2[:rows[v], :], ps2[:rows[v], :], q2v[:rows[v], :])
                        s2o = adw.tile([128, F], bf, tag="so", name="s2o")
                        nc.vector.scalar_tensor_tensor(
                            s2o[:rows[v], :], w2[:rows[v], :], dv[:rows[v], v:v + 1],
                            q1v[:rows[v], :], op0=MUL, op1=ADD)
                        nc.sync.dma_start(out=s_blk(s2_s, v), in_=s2o[:rows[v], :].rearrange("n (c t) -> n c t", t=Tp))

                    # pass 2: u = p0 + 0.5 (z1 + s1) + dinv (z2 + s2) + resid
                    uts = []
                    stats = adp_.tile([128, 16], fp, tag="stats")
                    nc.vector.memset(stats[:, :], 0.0)
                    for v in range(NT):
                        ps = aps.tile([128, F], fp, tag="psZ", name="psZ3")
                        sweep(aT, lambda k: s_blk(s1_s, k), v, ps)
                        s1v = adw.tile([128, F], fp, tag="in1", name="s1v")
                        nc.gpsimd.dma_start(out=s1v[:rows[v], :].rearrange("n (c t) -> n c t", t=Tp), in_=s_blk(s1_s, v))
                        p0v = adw.tile([128, F], fp, tag="in1", name="p0v")
                        nc.gpsimd.dma_start(out=p0v[:rows[v], :].rearrange("n (c t) -> n c t", t=Tp), in_=pall_blk(4, v))
                        w1 = adw.tile([128, F], fp, tag="w2", name="w1b")
                        nc.vector.tensor_add(w1[:rows[v], :], ps[:rows[v], :], s1v[:rows[v], :])
                        ut = adp_.tile([128, F], fp, tag="ut%d" % v, name="ut%d" % v)
                        nc.vector.scalar_tensor_tensor(
                            ut[:rows[v], :], w1[:rows[v], :], 0.5,
                            p0v[:rows[v], :], op0=MUL, op1=ADD)

                        ps2 = aps.tile([128, F], fp, tag="psZ", name="psZ4")
                        sweep(aD, lambda k: s_blk(s2_s, k), v, ps2)
                        s2v = adw.tile([128, F], fp, tag="in1", name="s2v")
                        nc.gpsimd.dma_start(out=s2v[:rows[v], :].rearrange("n (c t) -> n c t", t=Tp), in_=s_blk(s2_s, v))
                        w2 = adw.tile([128, F], fp, tag="w2", name="w2b")
                        nc.vector.tensor_add(w2[:rows[v], :], ps2[:rows[v], :], s2v[:rows[v], :])
                        nc.vector.scalar_tensor_tensor(
                            ut[:rows[v], :], w2[:rows[v], :], dv[:rows[v], v:v + 1],
                            ut[:rows[v], :], op0=MUL, op1=ADD)
                        # residual (last Tp of each channel of x_in)
                        xr = adw.tile([128, F], fp, tag="xr", name="xr")
                        nc.sync.dma_start(
                            out=xr[:rows[v], :].rearrange("n (c t) -> n c t", t=Tp),
                            in_=xin[off[v]:off[v] + rows[v], :].rearrange(
                                "n (c t) -> n c t", t=T)[:, :, T - Tp:])
                        nc.vector.tensor_add(ut[:rows[v], :], ut[:rows[v], :], xr[:rows[v], :])
                        # stats partials
                        nc.vector.reduce_sum(stats[:rows[v], 2 * v:2 * v + 1], ut[:rows[v], :], axis=AX)
                        usq = adw.tile([128, F], fp, tag="w2", name="usq")
                        nc.vector.tensor_mul(usq[:rows[v], :], ut[:rows[v], :], ut[:rows[v], :])
                        nc.vector.reduce_sum(stats[:rows[v], 2 * v + 1:2 * v + 2], usq[:rows[v], :], axis=AX)
                        uts.append(ut)

                    # layernorm stats across partitions
                    psS = aps.tile([1, 16], fp, tag="psS", name="psS")
                    nc.tensor.matmul(psS[:1, :16], onescol[:, :], stats[:, :], start=True, stop=True)
                    CNT = float(8 * N * Tp)
                    sm = adp_.tile([1, 8], fp, tag="sm")
                    nc.vector.reduce_sum(sm[:1, 0:1], psS[0:1, :].rearrange("p (v two) -> p two v", two=2)[:, 0:1, :], axis=AX)
                    nc.vector.reduce_sum(sm[:1, 1:2], psS[0:1, :].rearrange("p (v two) -> p two v", two=2)[:, 1:2, :], axis=AX)
                    # mu = sm0/CNT ; var = sm1/CNT - mu^2 ; inv = 1/sqrt(var+EPS)
                    nc.vector.tensor_scalar_mul(sm[:1, 2:3], sm[:1, 0:1], 1.0 / CNT)   # mu
                    nc.vector.tensor_scalar_mul(sm[:1, 3:4], sm[:1, 1:2], 1.0 / CNT)   # E[x^2]
                    nc.vector.tensor_mul(sm[:1, 4:5], sm[:1, 2:3], sm[:1, 2:3])        # mu^2
                    nc.vector.tensor_sub(sm[:1, 5:6], sm[:1, 3:4], sm[:1, 4:5])        # var
                    nc.vector.tensor_scalar_add(sm[:1, 5:6], sm[:1, 5:6], float(EPS))
                    nc.scalar.activation(sm[:1, 6:7], sm[:1, 5:6], AF.Sqrt)
                    nc.vector.reciprocal(sm[:1, 7:8], sm[:1, 6:7])                     # inv
                    nc.vector.tensor_scalar_mul(sm[:1, 2:3], sm[:1, 2:3], -1.0)        # -mu
                    bc2 = adp_.tile([1, 2], fp, tag="bc2")
                    nc.vector.tensor_copy(bc2[:1, 0:1], sm[:1, 2:3])
                    nc.vector.tensor_copy(bc2[:1, 1:2], sm[:1, 7:8])
                    psb = aps.tile([128, 2], fp, tag="psS", name="psb")
                    nc.tensor.matmul(psb[:, :2], onesrow[:, :], bc2[:1, :2], start=True, stop=True)
                    nc.scalar.activation(bc[:, :2], psb[:, :2], AF.Copy)

                    # apply LN, write x_{l+1} (or pooling input) to DRAM
                    for v in range(NT):
                        xo = adw.tile([128, F], fp, tag="xo", name="xo")
                        nc.vector.tensor_scalar(
                            xo[:rows[v], :], uts[v][:rows[v], :],
                            bc[:rows[v], 0:1], bc[:rows[v], 1:2], op0=ADD, op1=MUL)
                        if l < L - 1:
                            nc.sync.dma_start(out=xn[l + 1][off[v]:off[v] + rows[v], :], in_=xo[:rows[v], :])
                        else:
                            nc.sync.dma_start(out=xn[3][off[v]:off[v] + rows[v], :], in_=xo[:rows[v], :])

            # ---------------- adaptive avg pooling ----------------
            with tc.tile_pool(name="pool", bufs=2) as plp:
                bnds = _pool_bounds(TP[2])
                for v in range(NT):
                    xt = plp.tile([128, 8 * TP[2]], fp, tag="xt", name="xt")
                    nc.sync.dma_start(out=xt[:rows[v], :], in_=xn[3][off[v]:off[v] + rows[v], :])
                    ot = plp.tile([128, 8 * TSHORT], fp, tag="ot", name="ot")
                    xv = xt[:rows[v], :].rearrange("n (c t) -> n c t", t=TP[2])
                    for s, (a, w) in enumerate(bnds):
                        nc.vector.reduce_sum(ot[:rows[v], s * 8:(s + 1) * 8],
                                             xv[:, :, a:a + w], axis=AX)
                    nc.sync.dma_start(out=out_d[off[v]:off[v] + rows[v], :], in_=ot[:rows[v], :8 * TSHORT])
    return nc


def _device_forward(d, adp):
    try:
        import jax
        try:
            jax.config.update("jax_compilation_cache_dir", _CACHE_DIR)
            jax.config.update("jax_persistent_cache_min_compile_time_secs", 0.0)
            jax.config.update("jax_persistent_cache_min_entry_size_bytes", -1)
        except Exception:
            pass
        _install_birfix()
        from concourse.bass_utils import run_bass_kernel_spmd
        import ml_dtypes
        bf16 = ml_dtypes.bfloat16

        dinv = (1.0 / (1.0 + adp.sum(axis=0))).astype(f32)
        dpad = np.zeros((1024,), f32)
        dpad[:N] = dinv
        dmat = np.ascontiguousarray(dpad.reshape(8, 128).T)

        # start conv on host: x0 [B, N, 8, T]
        inp = d["input"]
        xf = inp.transpose(0, 2, 3, 1).reshape(-1, 2) @ d["start_w"].T
        xf += d["start_b"][None, :]
        x0 = np.ascontiguousarray(
            xf.reshape(B, N, TIN[0], 8).transpose(0, 1, 3, 2)).reshape(B, N, 8 * TIN[0])

        wts = _prep_weights(d)
        adpT_b = np.ascontiguousarray(adp.T).astype(bf16)
        adp_b = adp.astype(bf16)
        if _DEV["nc"] is None:
            _DEV["nc"] = _build_nc()
        in_maps = []
        for b in range(B):
            m = {"x0": x0[b].astype(f32), "adpT": adpT_b, "adp": adp_b,
                 "dinv": dmat}
            m.update(wts)
            in_maps.append(m)
        res = run_bass_kernel_spmd(_DEV["nc"], in_maps, list(range(B)))
        scale = np.array([1.0 / w for (_, w) in _pool_bounds(TP[2])], f32)
        outs = []
        for b in range(B):
            ob = np.asarray(res.results[b]["out"], f32).reshape(N, TSHORT, 8)
            outs.append(ob.transpose(2, 1, 0) * scale[None, :, None])
        return np.stack(outs, 0)
    except Exception:
        import traceback
        traceback.print_exc()
        _DEV["fail"] = True
        return None


# ---------------------------------------------------------------- host fallback
def _host_forward(d, adp):
    dinv = (1.0 / (1.0 + adp.sum(axis=0))).astype(f32)
    x = np.einsum("bint,oi->bont", d["input"], d["start_w"]).astype(f32) + \
        d["start_b"][None, :, None, None]
    for l in range(L):
        T = x.shape[-1]
        Tp = T - 7
        filts, gates = [], []
        for k in KSET:
            w, bias = d["fw%d" % k][l], d["fb%d" % k][l]
            kk = w.shape[-1]
            acc = np.zeros((B, w.shape[0], N, T - kk + 1), f32)
            for j in range(kk):
                acc += np.einsum("oi,bint->bont", w[:, :, 0, j],
                                 x[:, :, :, j:T - kk + 1 + j])
            filts.append((acc + bias[None, :, None, None])[..., -Tp:])
            w, bias = d["gw%d" % k][l], d["gb%d" % k][l]
            acc = np.zeros((B, w.shape[0], N, T - kk + 1), f32)
            for j in range(kk):
                acc += np.einsum("oi,bint->bont", w[:, :, 0, j],
                                 x[:, :, :, j:T - kk + 1 + j])
            gates.append((acc + bias[None, :, None, None])[..., -Tp:])
        filt = np.tanh(np.concatenate(filts, 1))
        gate = 1.0 / (1.0 + np.exp(-np.concatenate(gates, 1)))
        x1 = (filt * gate).astype(f32)
        Q0, Q1, Q2, R0, R1, R2, ub = _fold(d, l)
        p0 = np.einsum("oc,bcnt->bont", Q0 + R0, x1).astype(f32)
        p1 = np.einsum("oc,bcnt->bont", Q1, x1).astype(f32)
        p2 = np.einsum("oc,bcnt->bont", Q2, x1).astype(f32)
        q1 = np.einsum("oc,bcnt->bont", R1, x1).astype(f32)
        q2 = np.einsum("oc,bcnt->bont", R2, x1).astype(f32)
        z = np.einsum("vw,bowt->bovt", adp, p2)
        s1 = 0.5 * (z + p2) + p1
        z1 = np.einsum("vw,bowt->bovt", adp, s1)
        u = p0 + 0.5 * (z1 + s1)
        zz = np.einsum("wv,bowt->bovt", adp, q2)
        s1b = dinv[None, None, :, None] * (zz + q2) + q1
        zz1 = np.einsum("wv,bowt->bovt", adp, s1b)
        u = u + dinv[None, None, :, None] * (zz1 + s1b)
        u = u + ub[None, :, None, None].astype(f32) + x[:, :, :, -Tp:]
        mu = u.mean(axis=(1, 2, 3), keepdims=True)
        var = u.var(axis=(1, 2, 3), keepdims=True)
        x = ((u - mu) / np.sqrt(var + EPS)).astype(f32)
        x = x * d["nw%d" % (l + 1)][None] + d["nb%d" % (l + 1)][None]
    T = x.shape[-1]
    p = np.zeros((TSHORT, T), f32)
    for i in range(TSHORT):
        s = (i * T) // TSHORT
        e = -((-(i + 1) * T) // TSHORT)
        p[i, s:e] = 1.0 / (e - s)
    return np.einsum("st,bcnt->bcsn", p, x).astype(f32)


# ---------------------------------------------------------------- entry
_CHILD_KEYS = ("input", "start_w", "start_b",
               "fw2", "fw4", "fw6", "fw8", "fb2", "fb4", "fb6", "fb8",
               "gw2", "gw4", "gw6", "gw8", "gb2", "gb4", "gb6", "gb8",
               "g1_w", "g1_b", "g2_w", "g2_b")


def _child_main(inp, outp):
    z = np.load(inp)
    d = {k: z[k] for k in z.files if k != "adp"}
    out = _device_forward(d, z["adp"])
    if out is None:
        raise SystemExit(3)
    np.savez(outp, out=out)


def _device_subprocess(d, adp, grace=20.0):
    """Run the device path in a PYTHONHASHSEED=0 child (deterministic jax
    persistent-cache key — hash randomization perturbs it).  The axon device
    layer has a heavy-tailed session-establishment time (5 s .. minutes), so
    if the child is still running after `grace` seconds, compute the exact
    host fallback in parallel on this process and return whichever answer we
    have — capping the worst case near host speed.  The child is never
    killed (killing mid-device-call wedges the cores); it finishes and
    closes cleanly on its own."""
    import subprocess, sys, tempfile, time as _time, os as _os
    try:
        tmp = tempfile.mkdtemp()
        inp = _os.path.join(tmp, "in.npz")
        outp = _os.path.join(tmp, "out.npz")
        np.savez(inp, adp=adp, **{k: d[k] for k in _CHILD_KEYS})
        here = _os.path.dirname(_os.path.abspath(__file__))
        code = ("import sys; sys.path.insert(0, %r); "
                "import kernel; kernel._child_main(%r, %r)" % (here, inp, outp))
        env = dict(_os.environ)
        env["PYTHONHASHSEED"] = "0"
        proc = subprocess.Popen([sys.executable, "-c", code], env=env,
                                stdout=subprocess.DEVNULL,
                                stderr=subprocess.DEVNULL)
        deadline = _time.time() + grace
        while _time.time() < deadline:
            if proc.poll() is not None:
                break
            _time.sleep(0.1)
        if proc.poll() is None:
            # device session is slow — race it with the exact host path
            host_out = _host_forward(d, adp)
            return host_out
        if proc.returncode != 0 or not _os.path.exists(outp):
            return None
        return np.load(outp)["out"]
    except Exception:
        return None


def kernel(**d):
    import sys
    d = {k: np.asarray(v) for k, v in d.items()}
    adp = _graph_prep(d)
    ln_identity = all(
        bool(np.all(d["nw%d" % j] == 1.0)) and bool(np.all(d["nb%d" % j] == 0.0))
        for j in (1, 2, 3))
    if ln_identity and not _DEV["fail"]:
        out = None
        if sys.flags.hash_randomization == 0 or "jax" in sys.modules:
            out = _device_forward(d, adp)
        else:
            out = _device_subprocess(d, adp)
            if out is None:
                out = _device_forward(d, adp)
        if out is not None:
            return out
    return _host_forward(d, adp)
